# revision 1
# baseline (speedup 1.0000x reference)
"""Trainium2 kernel for nn_GaussianModel (gnn_message_passing).

Sharding: column blocks of the NxN matrices across 8 cores (each core owns
N/8 = 512 columns). The three matmul-heavy stages run on device:
  A) G = g @ g.T          (g = [m | cs], K=1024)  -> Gram col-blocks
  B) h = An.T @ Y1        (layer-1 aggregation)   -> row-blocks, no collective
  C) z = An.T @ Y2        (layer-2 aggregation, same NEFF as B)
Column sharding makes every stage collective-free: each core holds the full
stationary operand and one column block, producing disjoint output slices.
Elementwise chain runs on host with the exact reference formulas.
"""
import json
import sys
import time

sys.path.insert(0, "/opt/trn_rl_repo")
import numpy as np
import concourse.bass as bass
import concourse.mybir as mybir
from concourse.tile import TileContext
from concourse.bass_utils import run_bass_kernel_spmd

NC = 8
N, F, H = 4096, 512, 256
BLK = N // NC
EPS_CLAMP = 1e-6
f32, f16 = mybir.dt.float32, mybir.dt.float16

# ---------------------------------------------------------------------------
# walrus in this container caps sem-waits at 1 per instruction; Tile emits
# more. Split excess waits onto preceding same-engine Drains in the BIR JSON.
_MAX_WAITS = 1


def _fix_bir_bytes(bir_json):
    j = json.loads(bir_json)
    changed = False
    for fn in j.get("functions", []):
        for bb in fn.get("blocks", []):
            new_insts = []
            for inst in bb.get("instructions", []):
                si = inst.get("sync_info") or {}
                waits = si.get("on_wait") or []
                if len(waits) > _MAX_WAITS and inst.get("engine", "Unassigned") != "Unassigned":
                    changed = True
                    keep = waits[-_MAX_WAITS:]
                    extra = waits[:-_MAX_WAITS]
                    for gi in range(0, len(extra), _MAX_WAITS):
                        new_insts.append({
                            "debug": inst.get("debug", 0),
                            "engine": inst["engine"],
                            "ins": [],
                            "outs": [],
                            "name": f"{inst['name']}-ws{gi}",
                            "opcode": "Drain",
                            "sync_info": {"on_update": [],
                                          "on_wait": extra[gi:gi + _MAX_WAITS]},
                        })
                    si = dict(si)
                    si["on_wait"] = keep
                    inst = dict(inst)
                    inst["sync_info"] = si
                new_insts.append(inst)
            bb["instructions"] = new_insts
    return json.dumps(j).encode() if changed else bir_json


def _install_birfix():
    import concourse.bass_utils as bu
    if getattr(bu, "_birfix_installed", False):
        return
    orig = bu.compile_bir_kernel

    def patched(bir_json, tmpdir, neff_name="file.neff"):
        try:
            bir_json = _fix_bir_bytes(bir_json)
        except Exception as e:
            print("birfix failed:", e)
        return orig(bir_json, tmpdir, neff_name=neff_name)

    bu.compile_bir_kernel = patched
    try:
        import concourse.bass2jax as b2j
        b2j.compile_bir_kernel = patched
    except Exception as e:
        print("birfix bass2jax hook failed:", e)
    bu._birfix_installed = True


_install_birfix()

# ---------------------------------------------------------------------------
# Device kernels. Both are "C_colblock = LT.T-slices @ RB" style SPMD matmuls
# with fp16 inputs and fp32 PSUM accumulation; each core writes a disjoint
# output slice, so no collectives are needed.
_CACHE = {}
_LAST_DEVICE_WALL = 0.0


def _build_gram():
    # OUT[:, blk] for blk = this core: [4096, 512] = g @ g_blk.T
    # LT = g.T full [1024, 4096]; RB = g.T[:, blk] [1024, 512]
    nc = bass.Bass("TRN2", num_devices=NC)
    LT = nc.dram_tensor("LT", [1024, N], f16, kind="ExternalInput")
    RB = nc.dram_tensor("RB", [1024, BLK], f16, kind="ExternalInput")
    OUT = nc.dram_tensor("OUT", [N, BLK], f32, kind="ExternalOutput")
    with TileContext(nc) as tc:
        with (
            tc.tile_pool(name="sb", bufs=4) as sb,
            tc.tile_pool(name="rp", bufs=1) as rp,
            tc.tile_pool(name="ps", bufs=4, space="PSUM") as ps,
        ):
            rhs = rp.tile([128, 8, BLK], f16)
            for kc in range(8):
                nc.sync.dma_start(rhs[:, kc, :], RB[kc * 128:(kc + 1) * 128, :])
            for ic in range(N // 128):
                lt = sb.tile([128, 8, 128], f16, tag="lt")
                nc.sync.dma_start(
                    lt[:],
                    LT.ap().rearrange("(c p) n -> p c n", p=128)[:, :, ic * 128:(ic + 1) * 128],
                )
                acc = ps.tile([128, BLK], f32, tag="acc")
                for kc in range(8):
                    nc.tensor.matmul(acc[:], lt[:, kc, :], rhs[:, kc, :],
                                     start=(kc == 0), stop=(kc == 7))
                o = sb.tile([128, BLK], f32, tag="o")
                nc.scalar.copy(o[:], acc[:])
                nc.sync.dma_start(OUT[ic * 128:(ic + 1) * 128, :], o[:])
    return nc


def _build_agg():
    # out rows blk: [512, 1024] = An[:, blk].T @ Y   (Y = [Y_a | Y_b] packed)
    # AB = An col-block [4096, 512]; YF = Y full [4096, 1024]
    nc = bass.Bass("TRN2", num_devices=NC)
    AB = nc.dram_tensor("AB", [N, BLK], f16, kind="ExternalInput")
    YF = nc.dram_tensor("YF", [N, 1024], f16, kind="ExternalInput")
    OUT = nc.dram_tensor("OUT", [BLK, 1024], f32, kind="ExternalOutput")
    with TileContext(nc) as tc:
        with (
            tc.tile_pool(name="sb", bufs=4) as sb,
            tc.tile_pool(name="ap_", bufs=1) as apool,
            tc.tile_pool(name="ps", bufs=1, space="PSUM") as ps,
        ):
            an = apool.tile([128, 32, BLK], f16)
            for kc in range(32):
                nc.sync.dma_start(an[:, kc, :], AB[kc * 128:(kc + 1) * 128, :])
            accs = []
            for m in range(4):
                acc_m = ps.tile([128, 1024], f32, tag=f"acc{m}", name=f"acc{m}")
                accs.append(acc_m)
            for kc in range(32):
                y = sb.tile([128, 1024], f16, tag="y")
                nc.sync.dma_start(y[:], YF[kc * 128:(kc + 1) * 128, :])
                for m in range(4):  # output row chunks (j within block)
                    for nn in range(2):  # N chunks of 512
                        nc.tensor.matmul(
                            accs[m][:, nn * 512:(nn + 1) * 512],
                            an[:, kc, m * 128:(m + 1) * 128],
                            y[:, nn * 512:(nn + 1) * 512],
                            start=(kc == 0), stop=(kc == 31),
                        )
            for m in range(4):
                o = sb.tile([128, 1024], f32, tag="o")
                nc.scalar.copy(o[:], accs[m][:])
                nc.sync.dma_start(OUT[m * 128:(m + 1) * 128, :], o[:])
    return nc


def _run(name, builder, in_maps):
    global _LAST_DEVICE_WALL
    if name not in _CACHE:
        _CACHE[name] = builder()
    t0 = time.time()
    res = run_bass_kernel_spmd(_CACHE[name], in_maps, core_ids=list(range(NC)))
    _LAST_DEVICE_WALL += time.time() - t0
    return res.results


def _dev_gram(g16):
    gT = np.ascontiguousarray(g16.T)
    maps = [{"LT": gT, "RB": np.ascontiguousarray(gT[:, k * BLK:(k + 1) * BLK])}
            for k in range(NC)]
    res = _run("gram", _build_gram, maps)
    return np.concatenate([res[k]["OUT"] for k in range(NC)], axis=1)


def _dev_agg(An16, Y16):
    maps = [{"AB": np.ascontiguousarray(An16[:, k * BLK:(k + 1) * BLK]), "YF": Y16}
            for k in range(NC)]
    res = _run("agg", _build_agg, maps)
    return np.concatenate([res[k]["OUT"] for k in range(NC)], axis=0)


# ---------------------------------------------------------------------------
def _l2n(x):
    n = np.sqrt(np.sum(x * x, axis=1, keepdims=True))
    return x / np.maximum(n, 1e-12)


def kernel(x, new_edge, beta, delta, eps, Wm, bm, Ws, bs,
           mW0, mb0, mW1, mb1, sW0, sb0, sW1, sb1):
    global _LAST_DEVICE_WALL
    _LAST_DEVICE_WALL = 0.0
    x = np.asarray(x, np.float32)
    b = float(np.asarray(beta).reshape(-1)[0])
    d = float(np.asarray(delta).reshape(-1)[0])

    x_mean = x @ Wm + bm
    x_std = x @ Ws + bs

    m = _l2n(x_mean)
    c = _l2n(np.exp(x_std))
    cs = np.sqrt(c)
    sq = np.sum(m * m, axis=1)
    csum = np.sum(c, axis=1)

    g16 = np.concatenate([m, cs], axis=1).astype(np.float16)  # [N, 1024]
    G = _dev_gram(g16)  # m@m.T + cs@cs.T, fp16 inputs, fp32 accum

    u = sq + csum
    res = (u[:, None] + u[None, :]) - 2.0 * G
    ws = np.exp(-res)
    ws = _l2n(ws)

    term = (1.0 - b) * ws + b * np.asarray(new_edge, np.float32)
    term = np.clip(term, EPS_CLAMP, 1.0 - EPS_CLAMP)
    term = np.log(term / (1.0 - term))
    e = np.clip(np.asarray(eps, np.float32), EPS_CLAMP, 1.0 - EPS_CLAMP)
    term = term + np.log(e / (1.0 - e))
    term = 1.0 / (1.0 + np.exp(-term))
    A = np.where(term > d, term, 0.0)

    # GCN normalization (shared by all four conv calls)
    diag = np.diagonal(A).copy()
    A[np.arange(N), np.arange(N)] = np.where(diag > 0, diag, 1.0)
    deg = A.sum(axis=0)
    dis = np.where(deg > 0, deg ** -0.5, 0.0)
    An = (dis[:, None] * A * dis[None, :]).astype(np.float16)

    # layer 1 (mean + std packed)
    Y1 = np.concatenate([x_mean @ mW0, x_std @ sW0], axis=1).astype(np.float16)
    h = _dev_agg(An, Y1)
    h1m = np.maximum(h[:, :2 * H] + mb0, 0.0)
    h1s = np.maximum(h[:, 2 * H:] + sb0, 0.0)

    # layer 2 (mean + std packed, padded to 1024 to reuse the same NEFF)
    Y2 = np.zeros((N, 1024), np.float16)
    Y2[:, :H] = (h1m @ mW1).astype(np.float16)
    Y2[:, 512:512 + H] = (h1s @ sW1).astype(np.float16)
    z = _dev_agg(An, Y2)
    z_mean = np.maximum(z[:, :H] + mb1, 0.0).astype(np.float32)
    z_std = np.maximum(z[:, 512:512 + H] + sb1, 0.0).astype(np.float32)
    return z_mean, z_std



# revision 18
# speedup vs baseline: 3.2805x; 3.2805x over previous
"""Trainium2 kernel for nn_GaussianModel (gnn_message_passing).

Single fused device call, row-sharded across 8 cores (core k owns rows
r_k = [k*512, (k+1)*512) of the 4096-node graph):

  host:   x_mean/x_std projections, l2-norms, Y1 = [xm@mW0 | xs@sW0],
          E = e/(1-e), b*new_edge  (cheap O(N*F) / elementwise prep)
  device: Gram via augmented-K matmul  q_ij = g_i.g_j - (u_i+u_j)/2
          ws = exp(2q), row-l2-norm, logit-mix with new_edge/eps
          (algebraic sigmoid form, no log/exp), threshold -> A row-block
          column degree partials -> ReduceScatter -> dis = deg^-1/2
          layer-1 aggregation partials A_k^T @ (dis*Y1) -> ReduceScatter
          relu(+b0), on-device h @ blockdiag(mW1,sW1) via PE transpose
          layer-2 aggregation partials -> ReduceScatter -> relu(+b1)
  The GCN self-loop (add 1 where diag==0) is handled analytically: a
  per-row delta in {0,1} computed from host-supplied diagonal scalars,
  added to deg and applied as a rank-1 correction after each RS.

All per-core variation lives in the input data (no partition-id control
flow); collectives (2x AllGather, 3x ReduceScatter) handle placement.
"""
import json
import sys
import time

sys.path.insert(0, "/opt/trn_rl_repo")
import numpy as np
import concourse.bass as bass
import concourse.mybir as mybir
from concourse.tile import TileContext
from concourse.bass_utils import run_bass_kernel_spmd

NC = 8
N, F, H = 4096, 512, 256
BLK = N // NC  # 512 rows per core
EPS_CLAMP = 1e-6
f32, f16 = mybir.dt.float32, mybir.dt.float16
AF = mybir.ActivationFunctionType
OP = mybir.AluOpType

# ---------------------------------------------------------------------------
# walrus in this container caps sem-waits at 1 per instruction; Tile emits
# more. Split excess waits onto preceding same-engine Drains in the BIR JSON.
_MAX_WAITS = 1


def _fix_bir_bytes(bir_json):
    j = json.loads(bir_json)
    changed = False
    for fn in j.get("functions", []):
        for bb in fn.get("blocks", []):
            new_insts = []
            for inst in bb.get("instructions", []):
                si = inst.get("sync_info") or {}
                waits = si.get("on_wait") or []
                if len(waits) > _MAX_WAITS and inst.get("engine", "Unassigned") != "Unassigned":
                    changed = True
                    keep = waits[-_MAX_WAITS:]
                    extra = waits[:-_MAX_WAITS]
                    for gi in range(0, len(extra), _MAX_WAITS):
                        new_insts.append({
                            "debug": inst.get("debug", 0),
                            "engine": inst["engine"],
                            "ins": [],
                            "outs": [],
                            "name": f"{inst['name']}-ws{gi}",
                            "opcode": "Drain",
                            "sync_info": {"on_update": [],
                                          "on_wait": extra[gi:gi + _MAX_WAITS]},
                        })
                    si = dict(si)
                    si["on_wait"] = keep
                    inst = dict(inst)
                    inst["sync_info"] = si
                new_insts.append(inst)
            bb["instructions"] = new_insts
    return json.dumps(j).encode() if changed else bir_json


def _install_birfix():
    import concourse.bass_utils as bu
    if getattr(bu, "_birfix_installed", False):
        return
    orig = bu.compile_bir_kernel

    def patched(bir_json, tmpdir, neff_name="file.neff"):
        try:
            bir_json = _fix_bir_bytes(bir_json)
        except Exception as e:
            print("birfix failed:", e)
        return orig(bir_json, tmpdir, neff_name=neff_name)

    bu.compile_bir_kernel = patched
    try:
        import concourse.bass2jax as b2j
        b2j.compile_bir_kernel = patched
    except Exception as e:
        print("birfix bass2jax hook failed:", e)
    bu._birfix_installed = True


_install_birfix()


# ---------------------------------------------------------------------------
# run_bass_via_pjrt rebuilds jax.jit + recompiles the NEFF on EVERY call.
# Memoize the jitted sharded executable per Bass object so warm calls only
# pay input transfer + device execution.
def _install_pjrt_cache():
    import concourse.bass2jax as b2j

    if getattr(b2j, "_pjrt_cache_installed", False):
        return
    import jax
    from jax.sharding import Mesh, PartitionSpec
    from jax.experimental.shard_map import shard_map

    _runners = {}

    def _build_runner(nc, n_cores):
        b2j.install_neuronx_cc_hook()
        partition_name = (
            nc.partition_id_tensor.name if nc.partition_id_tensor else None
        )
        in_names, out_names, out_avals = [], [], []
        for alloc in nc.m.functions[0].allocations:
            if not isinstance(alloc, mybir.MemoryLocationSet):
                continue
            name = alloc.memorylocations[0].name
            if alloc.kind == "ExternalInput":
                if name != partition_name:
                    in_names.append(name)
            elif alloc.kind == "ExternalOutput":
                out_names.append(name)
                out_avals.append(
                    jax.core.ShapedArray(
                        tuple(alloc.tensor_shape), mybir.dt.np(alloc.dtype)
                    )
                )
        n_params = len(in_names)
        n_outs = len(out_avals)
        all_in = in_names + out_names + ([partition_name] if partition_name else [])
        donate = tuple(range(n_params, n_params + n_outs))

        def _body(*args):
            operands = list(args)
            if partition_name is not None:
                operands.append(b2j.partition_id_tensor())
            outs = b2j._bass_exec_p.bind(
                *operands,
                out_avals=tuple(out_avals),
                in_names=tuple(all_in),
                out_names=tuple(out_names),
                lowering_input_output_aliases=(),
                sim_require_finite=True,
                sim_require_nnan=True,
                nc=nc,
            )
            return tuple(outs)

        devices = jax.devices()[:n_cores]
        mesh = Mesh(np.asarray(devices), ("core",))
        sharded = jax.jit(
            shard_map(
                _body,
                mesh=mesh,
                in_specs=(PartitionSpec("core"),) * (n_params + n_outs),
                out_specs=(PartitionSpec("core"),) * n_outs,
                check_rep=False,
            ),
            donate_argnums=donate,
            keep_unused=True,
        )

        def run(in_maps):
            if nc.dbg_addr is not None:
                in_maps = [
                    {**m, nc.dbg_addr.name: np.zeros((1, 2), np.uint32)}
                    for m in in_maps
                ]
            per_core = [[np.asarray(m[name]) for name in in_names] for m in in_maps]
            concat_in = [
                np.concatenate([per_core[c][i] for c in range(n_cores)], axis=0)
                for i in range(n_params)
            ]
            concat_zeros = [
                np.zeros((n_cores * av.shape[0], *av.shape[1:]), av.dtype)
                for av in out_avals
            ]
            out_arrs = sharded(*concat_in, *concat_zeros)
            return [
                {
                    name: np.asarray(out_arrs[i]).reshape(
                        n_cores, *out_avals[i].shape
                    )[c]
                    for i, name in enumerate(out_names)
                }
                for c in range(n_cores)
            ]

        return run

    def cached(nc, in_maps, n_cores):
        key = (id(nc), n_cores)
        if key not in _runners:
            _runners[key] = _build_runner(nc, n_cores)
        return _runners[key](in_maps)

    b2j.run_bass_via_pjrt = cached
    b2j._pjrt_cache_installed = True


_install_pjrt_cache()

# ---------------------------------------------------------------------------
_CACHE = {}
_LAST_DEVICE_WALL = 0.0

NMT = 4   # row-tiles of 128 per core
NCC = 8   # column chunks of 512
NKC = 9   # K-tiles of 128 (1024 g-features + 4 augmented rows, padded)
KAUG = 1152  # 9 * 128


def _build_fused():
    nc = bass.Bass("TRN2", num_devices=NC)
    RK = nc.dram_tensor("RK", [KAUG, BLK], f16, kind="ExternalInput")
    LA = nc.dram_tensor("LA", [4, BLK], f16, kind="ExternalInput")
    NEB = nc.dram_tensor("NEB", [BLK, N], f16, kind="ExternalInput")
    EF = nc.dram_tensor("EF", [BLK, N], f16, kind="ExternalInput")
    Y1I = nc.dram_tensor("Y1I", [BLK, 1024], f16, kind="ExternalInput")
    WBK = nc.dram_tensor("WBK", [128, 512], f16, kind="ExternalInput")
    BR = nc.dram_tensor("BR", [1, 1536], f16, kind="ExternalInput")
    SM = nc.dram_tensor("SM", [128, 16], f32, kind="ExternalInput")
    ZOUT = nc.dram_tensor("ZOUT", [BLK, 512], f32, kind="ExternalOutput")

    identity_np = np.eye(128, dtype=np.float16)
    ones_col_np = np.ones((128, 1), dtype=np.float16)
    ones_row_np = np.ones((1, 128), dtype=np.float16)
    IDC = nc.inline_tensor(identity_np, name="idc")
    OCC = nc.inline_tensor(ones_col_np, name="occ")
    ORC = nc.inline_tensor(ones_row_np, name="orc")

    rg = [list(range(NC))]

    with TileContext(nc) as tc:
        with (
            tc.tile_pool(name="sb", bufs=1) as sb,
            tc.tile_pool(name="rfp", bufs=2) as rfp,
            tc.tile_pool(name="io", bufs=3) as io,
            tc.tile_pool(name="ps", bufs=4, space="PSUM") as ps,
            tc.tile_pool(name="dr", bufs=1, space="DRAM") as dr,
        ):
            # ---------------- DRAM scratch + collectives wiring ------------
            rgin = dr.tile([KAUG, BLK], f16)
            rgout = dr.tile([NC, KAUG, BLK], f16, addr_space="Shared")
            wbin = dr.tile([128, 512], f16)
            wbout = dr.tile([NC, 128, 512], f16, addr_space="Shared")
            degb = dr.tile([N], f32)
            degs = dr.tile([BLK], f32)
            pb = dr.tile([N, 1024], f16)
            rs1 = dr.tile([BLK, 1024], f16)
            p2b = dr.tile([N, 512], f16)
            rs2 = dr.tile([BLK, 512], f16)

            nc.sync.dma_start(rgin[:], RK[:])
            nc.gpsimd.collective_compute(
                "AllGather", OP.bypass, replica_groups=rg,
                ins=[rgin[:].opt()], outs=[rgout[:].opt()],
            )
            nc.sync.dma_start(wbin[:], WBK[:])
            nc.gpsimd.collective_compute(
                "AllGather", OP.bypass, replica_groups=rg,
                ins=[wbin[:].opt()], outs=[wbout[:].opt()],
            )

            # ---------------- constants / scalars --------------------------
            ident = sb.tile([128, 128], f16)
            nc.sync.dma_start(ident[:], IDC[:])
            onescol = sb.tile([128, 1], f16)
            nc.sync.dma_start(onescol[:], OCC[:])
            onesrow = sb.tile([1, 128], f16)
            nc.sync.dma_start(onesrow[:], ORC[:])
            sm = sb.tile([128, 16], f32)
            nc.sync.dma_start(sm[:], SM[:])
            brs = sb.tile([1, 1536], f16)
            nc.sync.dma_start(brs[:], BR[:])

            # bias broadcast via K=1 matmul: ones_col x bias_row
            b0bc = sb.tile([128, 1024], f32)
            b1bc = sb.tile([128, 512], f32)
            for j in range(3):
                psb = ps.tile([128, 512], f32, tag="ps", name=f"psb{j}")
                nc.tensor.matmul(psb[:], onesrow[:], brs[:, j * 512:(j + 1) * 512],
                                 start=True, stop=True)
                dst = b0bc[:, j * 512:(j + 1) * 512] if j < 2 else b1bc[:]
                nc.scalar.copy(dst, psb[:])

            # ---------------- lhsT for the gram stage ----------------------
            # lk[:, kc, :] = RK[kc*128:(kc+1)*128, :]; K-tile 8 swaps the
            # augmented rows: lhsT aug = [vhi; vlo; 1; 1] vs rhs [1; 1; vhi; vlo]
            lk = sb.tile([128, NKC, BLK], f16)
            for kc in range(8):
                nc.sync.dma_start(lk[:, kc, :], RK[kc * 128:(kc + 1) * 128, :])
            nc.vector.memset(lk[:, 8, :], 0.0)
            nc.sync.dma_start(lk[0:4, 8, :], LA[:])

            # ---------------- gram + ws = exp(2q) ---------------------------
            # wst holds ws (f16) per (mt, c) chunk; the elementwise chain
            # later overwrites each chunk in place with the final A values.
            wst = sb.tile([128, NMT, NCC, 512], f16)
            sqa = sb.tile([128, 32], f32)
            for c in range(NCC):
                rf = rfp.tile([128, NKC, 512], f16, tag="rf")
                for kc in range(NKC):
                    nc.sync.dma_start(
                        rf[:, kc, :], rgout[c, kc * 128:(kc + 1) * 128, :])
                for mt in range(NMT):
                    psg = ps.tile([128, 512], f32, tag="ps", name=f"psg{c}_{mt}")
                    for kc in range(NKC):
                        nc.tensor.matmul(
                            psg[:], lk[:, kc, mt * 128:(mt + 1) * 128],
                            rf[:, kc, :], start=(kc == 0), stop=(kc == NKC - 1))
                    wsl = wst[:, mt, c, :]
                    nc.scalar.activation(wsl, psg[:], AF.Exp, scale=2.0)
                    junk = io.tile([128, 512], f16, tag="junk", bufs=2)
                    nc.scalar.activation(
                        junk[:], wsl, AF.Square,
                        accum_out=sqa[:, mt * 8 + c:mt * 8 + c + 1])

            # ---------------- row norms + delta (self-loop indicator) ------
            rn4 = sb.tile([128, 4], f32)     # 1/||ws_row||
            rnb4 = sb.tile([128, 4], f32)    # (1-b) * rn4
            dl4 = sb.tile([128, 4], f32)     # delta in {0,1}
            for mt in range(NMT):
                rtmp = io.tile([128, 1], f32, tag="rtmp")
                nc.vector.tensor_reduce(
                    rtmp[:], sqa[:, mt * 8:(mt + 1) * 8],
                    axis=mybir.AxisListType.X, op=OP.add)
                rsq = io.tile([128, 1], f32, tag="rsq")
                nc.scalar.sqrt(rsq[:], rtmp[:])
                nc.vector.reciprocal(rn4[:, mt:mt + 1], rsq[:])
                nc.vector.tensor_scalar(
                    out=rnb4[:, mt:mt + 1], in0=rn4[:, mt:mt + 1],
                    scalar1=sm[:, 13:14], scalar2=None, op0=OP.mult)
                # delta chain on [128,1] diagonal scalars
                wsn = io.tile([128, 1], f32, tag="wsn")
                nc.vector.tensor_tensor(
                    out=wsn[:], in0=sm[:, mt:mt + 1], in1=rn4[:, mt:mt + 1],
                    op=OP.mult)
                t1d = io.tile([128, 1], f32, tag="t1d")
                nc.vector.scalar_tensor_tensor(
                    out=t1d[:], in0=wsn[:], scalar=sm[:, 13:14],
                    in1=sm[:, 4 + mt:5 + mt], op0=OP.mult, op1=OP.add)
                nc.vector.tensor_scalar(
                    out=t1d[:], in0=t1d[:], scalar1=1.0 - EPS_CLAMP,
                    scalar2=EPS_CLAMP, op0=OP.min, op1=OP.max)
                pd = io.tile([128, 1], f32, tag="pd")
                nc.vector.tensor_tensor(
                    out=pd[:], in0=t1d[:], in1=sm[:, 8 + mt:9 + mt], op=OP.mult)
                ndd = io.tile([128, 1], f32, tag="ndd")
                nc.vector.scalar_tensor_tensor(
                    out=ndd[:], in0=t1d[:], scalar=1.0, in1=pd[:],
                    op0=OP.subtract, op1=OP.subtract)
                rdd = io.tile([128, 1], f32, tag="rdd")
                nc.vector.reciprocal(rdd[:], ndd[:])
                qdd = io.tile([128, 1], f32, tag="qdd")
                nc.vector.tensor_tensor(
                    out=qdd[:], in0=pd[:], in1=rdd[:], op=OP.mult)
                nc.vector.tensor_scalar(
                    out=dl4[:, mt:mt + 1], in0=qdd[:], scalar1=sm[:, 12:13],
                    scalar2=None, op0=OP.is_ge)

            # ---------------- elementwise chain -> A (f16, in wst) ----------
            # term = clip((1-b)*ws_n + b*ne); A = sig(logit(term)+logit(e))
            # with sig(logit(t)+logit(e)) = t*E/(t*E + 1 - t),  E = e/(1-e)
            for c in range(NCC):
                for mt in range(NMT):
                    neb = io.tile([128, 512], f16, tag="neb")
                    nc.sync.dma_start(
                        neb[:], NEB[mt * 128:(mt + 1) * 128, c * 512:(c + 1) * 512])
                    eft = io.tile([128, 512], f16, tag="eft")
                    nc.sync.dma_start(
                        eft[:], EF[mt * 128:(mt + 1) * 128, c * 512:(c + 1) * 512])
                    efc = io.tile([128, 512], f32, tag="efc")
                    nc.scalar.copy(efc[:], eft[:])
                    t1 = io.tile([128, 512], f32, tag="t1")
                    nc.vector.scalar_tensor_tensor(
                        out=t1[:], in0=wst[:, mt, c, :], scalar=rnb4[:, mt:mt + 1],
                        in1=neb[:], op0=OP.mult, op1=OP.add)
                    nc.vector.tensor_scalar(
                        out=t1[:], in0=t1[:], scalar1=1.0 - EPS_CLAMP,
                        scalar2=EPS_CLAMP, op0=OP.min, op1=OP.max)
                    pt = io.tile([128, 512], f32, tag="pt")
                    nc.vector.tensor_tensor(
                        out=pt[:], in0=t1[:], in1=efc[:], op=OP.mult)
                    nd = io.tile([128, 512], f32, tag="efc")
                    nc.vector.scalar_tensor_tensor(
                        out=nd[:], in0=t1[:], scalar=1.0, in1=pt[:],
                        op0=OP.subtract, op1=OP.subtract)
                    rc = io.tile([128, 512], f32, tag="t1")
                    nc.vector.reciprocal(rc[:], nd[:])
                    q1 = io.tile([128, 512], f32, tag="pt")
                    nc.vector.tensor_tensor(
                        out=q1[:], in0=pt[:], in1=rc[:], op=OP.mult)
                    msk = io.tile([128, 512], f32, tag="msk")
                    nc.vector.tensor_scalar(
                        out=msk[:], in0=q1[:], scalar1=sm[:, 12:13],
                        scalar2=None, op0=OP.is_lt)
                    nc.vector.scalar_tensor_tensor(
                        out=wst[:, mt, c, :], in0=q1[:], scalar=-1.0,
                        in1=msk[:], op0=OP.mult, op1=OP.mult)

            # ---------------- column-degree partials -> RS ------------------
            for c in range(NCC):
                psd = ps.tile([1, 512], f32, tag="ps", name=f"psd{c}")
                for mt in range(NMT):
                    nc.tensor.matmul(psd[:], onescol[:], wst[:, mt, c, :],
                                     start=(mt == 0), stop=(mt == NMT - 1))
                degc = io.tile([1, 512], f32, tag="degc", bufs=2)
                nc.scalar.copy(degc[:], psd[:])
                nc.sync.dma_start(degb[c * 512:(c + 1) * 512], degc[:])
            nc.gpsimd.collective_compute(
                "ReduceScatter", OP.add, replica_groups=rg,
                ins=[degb[:].opt()], outs=[degs[:].opt()],
            )
            degl = sb.tile([128, 4], f32)
            nc.sync.dma_start(degl[:], degs.rearrange("(t p) -> p t", p=128))
            degf = sb.tile([128, 4], f32)
            nc.vector.tensor_tensor(out=degf[:], in0=degl[:], in1=dl4[:], op=OP.add)
            dsq = sb.tile([128, 4], f32)
            nc.scalar.sqrt(dsq[:], degf[:])
            dis4 = sb.tile([128, 4], f32)
            nc.vector.reciprocal(dis4[:], dsq[:])

            # ---------------- Y1 scaled ------------------------------------
            y1s = sb.tile([128, NMT, 1024], f16)
            for mt in range(NMT):
                nc.sync.dma_start(y1s[:, mt, :], Y1I[mt * 128:(mt + 1) * 128, :])
                nc.scalar.activation(y1s[:, mt, :], y1s[:, mt, :], AF.Copy,
                                     scale=dis4[:, mt:mt + 1])

            # ---------------- layer-1 aggregation partials -> RS ------------
            for c in range(NCC):
                for w in range(4):
                    psa = ps.tile([128, 1024], f32, tag="ps", name=f"psa{c}_{w}")
                    for mt in range(NMT):
                        lhsT = wst[:, mt, c, w * 128:(w + 1) * 128]
                        for nn in range(2):
                            nc.tensor.matmul(
                                psa[:, nn * 512:(nn + 1) * 512], lhsT,
                                y1s[:, mt, nn * 512:(nn + 1) * 512],
                                start=(mt == 0), stop=(mt == NMT - 1))
                    o1 = io.tile([128, 1024], f16, tag="o1")
                    nc.scalar.copy(o1[:], psa[:])
                    nc.sync.dma_start(
                        pb[c * 512 + w * 128:c * 512 + (w + 1) * 128, :], o1[:])
            nc.gpsimd.collective_compute(
                "ReduceScatter", OP.add, replica_groups=rg,
                ins=[pb[:].opt()], outs=[rs1[:].opt()],
            )

            # ---------------- layer-1 post: h = relu(dis*(S + delta*Y1s) + b0)
            h16 = sb.tile([128, NMT, 1024], f16)
            for mt in range(NMT):
                rst = io.tile([128, 1024], f16, tag="rst", bufs=2)
                nc.sync.dma_start(rst[:], rs1[mt * 128:(mt + 1) * 128, :])
                s1 = io.tile([128, 1024], f32, tag="s1", bufs=2)
                nc.vector.scalar_tensor_tensor(
                    out=s1[:], in0=y1s[:, mt, :], scalar=dl4[:, mt:mt + 1],
                    in1=rst[:], op0=OP.mult, op1=OP.add)
                nc.vector.scalar_tensor_tensor(
                    out=s1[:], in0=s1[:], scalar=dis4[:, mt:mt + 1],
                    in1=b0bc[:], op0=OP.mult, op1=OP.add)
                nc.scalar.activation(h16[:, mt, :], s1[:], AF.Relu)

            # ---------------- Y2 = (h @ blockdiag(mW1,sW1)) * dis -----------
            wbt = sb.tile([128, 8, 512], f16)
            for kb in range(8):
                nc.sync.dma_start(wbt[:, kb, :], wbout[kb, :, :])
            y2s = sb.tile([128, NMT, 512], f16)
            for mt in range(NMT):
                htb = io.tile([128, 8, 128], f16, tag="htb", bufs=2)
                for kb in range(8):
                    pst = ps.tile([128, 128], f16, tag="ps", name=f"pst{mt}_{kb}")
                    nc.tensor.transpose(
                        pst[:], h16[:, mt, kb * 128:(kb + 1) * 128], ident[:])
                    nc.scalar.copy(htb[:, kb, :], pst[:])
                psy = ps.tile([128, 512], f32, tag="ps", name=f"psy{mt}")
                for kb in range(8):
                    nc.tensor.matmul(psy[:], htb[:, kb, :], wbt[:, kb, :],
                                     start=(kb == 0), stop=(kb == 7))
                nc.scalar.activation(y2s[:, mt, :], psy[:], AF.Copy,
                                     scale=dis4[:, mt:mt + 1])

            # ---------------- layer-2 aggregation partials -> RS ------------
            for c in range(NCC):
                for w in range(4):
                    ps2 = ps.tile([128, 512], f32, tag="ps", name=f"ps2{c}_{w}")
                    for mt in range(NMT):
                        nc.tensor.matmul(
                            ps2[:], wst[:, mt, c, w * 128:(w + 1) * 128],
                            y2s[:, mt, :], start=(mt == 0), stop=(mt == NMT - 1))
                    o2 = io.tile([128, 512], f16, tag="o2")
                    nc.scalar.copy(o2[:], ps2[:])
                    nc.sync.dma_start(
                        p2b[c * 512 + w * 128:c * 512 + (w + 1) * 128, :], o2[:])
            nc.gpsimd.collective_compute(
                "ReduceScatter", OP.add, replica_groups=rg,
                ins=[p2b[:].opt()], outs=[rs2[:].opt()],
            )

            # ---------------- layer-2 post + output -------------------------
            for mt in range(NMT):
                r2t = io.tile([128, 512], f16, tag="r2t", bufs=2)
                nc.sync.dma_start(r2t[:], rs2[mt * 128:(mt + 1) * 128, :])
                z1 = io.tile([128, 512], f32, tag="z1", bufs=2)
                nc.vector.scalar_tensor_tensor(
                    out=z1[:], in0=y2s[:, mt, :], scalar=dl4[:, mt:mt + 1],
                    in1=r2t[:], op0=OP.mult, op1=OP.add)
                nc.vector.scalar_tensor_tensor(
                    out=z1[:], in0=z1[:], scalar=dis4[:, mt:mt + 1],
                    in1=b1bc[:], op0=OP.mult, op1=OP.add)
                zo = io.tile([128, 512], f32, tag="zo", bufs=2)
                nc.scalar.activation(zo[:], z1[:], AF.Relu)
                nc.sync.dma_start(ZOUT[mt * 128:(mt + 1) * 128, :], zo[:])
    return nc


def _run(name, builder, in_maps):
    global _LAST_DEVICE_WALL
    if name not in _CACHE:
        _CACHE[name] = builder()
    t0 = time.time()
    res = run_bass_kernel_spmd(_CACHE[name], in_maps, core_ids=list(range(NC)))
    dt = time.time() - t0
    _LAST_DEVICE_WALL += dt
    import os
    if os.environ.get("KERNEL_DEBUG_TIMES"):
        print(f"  [dev call {name}: {dt * 1e3:.1f} ms]")
    return res.results


# ---------------------------------------------------------------------------
def _l2n(x):
    n = np.sqrt(np.sum(x * x, axis=1, keepdims=True))
    return x / np.maximum(n, 1e-12)


def kernel(x, new_edge, beta, delta, eps, Wm, bm, Ws, bs,
           mW0, mb0, mW1, mb1, sW0, sb0, sW1, sb1):
    global _LAST_DEVICE_WALL
    _LAST_DEVICE_WALL = 0.0
    x = np.asarray(x, np.float32)
    b = float(np.asarray(beta).reshape(-1)[0])
    d = float(np.asarray(delta).reshape(-1)[0])

    x_mean = x @ Wm + bm
    x_std = x @ Ws + bs

    m = _l2n(x_mean)
    c = _l2n(np.exp(x_std))
    cs = np.sqrt(c)
    sq = np.sum(m * m, axis=1)
    csum = np.sum(c, axis=1)
    u = (sq + csum).astype(np.float32)

    g = np.concatenate([m, cs], axis=1).astype(np.float32)  # [N, 1024]
    v = -0.5 * u
    vhi = v.astype(np.float16)
    vlo = (v - vhi.astype(np.float32)).astype(np.float16)

    # R (rhs side of the gram): rows 0:1024 g.T; 1024-25: ones; 1026-27: v
    R = np.zeros((KAUG, N), np.float16)
    R[0:1024] = g.T.astype(np.float16)
    R[1024] = 1.0
    R[1025] = 1.0
    R[1026] = vhi
    R[1027] = vlo

    ne = np.asarray(new_edge, np.float32)
    NEBf = (b * ne).astype(np.float16)
    e = np.clip(np.asarray(eps, np.float32), EPS_CLAMP, 1.0 - EPS_CLAMP)
    Ef = np.minimum(e / (1.0 - e), 60000.0).astype(np.float16)

    Y1 = np.concatenate([x_mean @ mW0, x_std @ sW0], axis=1).astype(np.float16)

    WBD = np.zeros((1024, 512), np.float16)
    WBD[:512, :256] = mW1
    WBD[512:, 256:] = sW1

    BRv = np.concatenate([mb0, sb0, mb1, sb1]).astype(np.float16).reshape(1, 1536)

    # host diagonal scalars (match the on-device dataflow, f32 approx)
    g2 = np.sum(g * g, axis=1)
    wsdiag = np.exp(-(2.0 * u - 2.0 * g2)).astype(np.float32)
    nebdiag = np.diagonal(NEBf).astype(np.float32)
    Ediag = np.diagonal(Ef).astype(np.float32)

    in_maps = []
    for k in range(NC):
        r0, r1 = k * BLK, (k + 1) * BLK
        smk = np.zeros((128, 16), np.float32)
        idx = np.arange(r0, r1)
        smk[:, 0:4] = wsdiag[idx].reshape(4, 128).T
        smk[:, 4:8] = nebdiag[idx].reshape(4, 128).T
        smk[:, 8:12] = Ediag[idx].reshape(4, 128).T
        smk[:, 12] = -d
        smk[:, 13] = 1.0 - b
        la = np.empty((4, BLK), np.float16)
        la[0] = vhi[r0:r1]
        la[1] = vlo[r0:r1]
        la[2] = 1.0
        la[3] = 1.0
        in_maps.append({
            "RK": np.ascontiguousarray(R[:, r0:r1]),
            "LA": la,
            "NEB": np.ascontiguousarray(NEBf[r0:r1]),
            "EF": np.ascontiguousarray(Ef[r0:r1]),
            "Y1I": np.ascontiguousarray(Y1[r0:r1]),
            "WBK": np.ascontiguousarray(WBD[k * 128:(k + 1) * 128]),
            "BR": BRv,
            "SM": smk,
        })

    res = _run("fused", _build_fused, in_maps)
    z = np.concatenate([res[k]["ZOUT"] for k in range(NC)], axis=0)
    z_mean = np.ascontiguousarray(z[:, :H]).astype(np.float32)
    z_std = np.ascontiguousarray(z[:, H:2 * H]).astype(np.float32)
    return z_mean, z_std


# revision 24
# speedup vs baseline: 7.6399x; 2.3288x over previous
"""Trainium2 kernel for nn_GaussianModel (gnn_message_passing).

Single fused device call, row-sharded across 8 cores (core k owns rows
r_k = [k*512, (k+1)*512) of the 4096-node graph):

  host:   x_mean/x_std projections, l2-norms, Y1 = [xm@mW0 | xs@sW0],
          E = e/(1-e), b*new_edge  (cheap O(N*F) / elementwise prep)
  device: Gram via augmented-K matmul  q_ij = g_i.g_j - (u_i+u_j)/2
          ws = exp(2q), row-l2-norm, logit-mix with new_edge/eps
          (algebraic sigmoid form, no log/exp), threshold -> A row-block
          column degree partials -> ReduceScatter -> dis = deg^-1/2
          layer-1 aggregation partials A_k^T @ (dis*Y1) -> ReduceScatter
          relu(+b0), on-device h @ blockdiag(mW1,sW1) via PE transpose
          layer-2 aggregation partials -> ReduceScatter -> relu(+b1)
  The GCN self-loop (add 1 where diag==0) is handled analytically: a
  per-row delta in {0,1} computed from host-supplied diagonal scalars,
  added to deg and applied as a rank-1 correction after each RS.

All per-core variation lives in the input data (no partition-id control
flow); collectives (2x AllGather, 3x ReduceScatter) handle placement.
"""
import json
import sys
import time

sys.path.insert(0, "/opt/trn_rl_repo")
import numpy as np
import concourse.bass as bass
import concourse.mybir as mybir
from concourse.tile import TileContext
from concourse.bass_utils import run_bass_kernel_spmd

NC = 8
N, F, H = 4096, 512, 256
BLK = N // NC  # 512 rows per core
EPS_CLAMP = 1e-6
f32, f16 = mybir.dt.float32, mybir.dt.float16
AF = mybir.ActivationFunctionType
OP = mybir.AluOpType

# ---------------------------------------------------------------------------
# walrus in this container caps sem-waits at 1 per instruction; Tile emits
# more. Split excess waits onto preceding same-engine Drains in the BIR JSON.
_MAX_WAITS = 1


def _fix_bir_bytes(bir_json):
    j = json.loads(bir_json)
    changed = False
    for fn in j.get("functions", []):
        for bb in fn.get("blocks", []):
            new_insts = []
            for inst in bb.get("instructions", []):
                si = inst.get("sync_info") or {}
                waits = si.get("on_wait") or []
                if len(waits) > _MAX_WAITS and inst.get("engine", "Unassigned") != "Unassigned":
                    changed = True
                    keep = waits[-_MAX_WAITS:]
                    extra = waits[:-_MAX_WAITS]
                    for gi in range(0, len(extra), _MAX_WAITS):
                        new_insts.append({
                            "debug": inst.get("debug", 0),
                            "engine": inst["engine"],
                            "ins": [],
                            "outs": [],
                            "name": f"{inst['name']}-ws{gi}",
                            "opcode": "Drain",
                            "sync_info": {"on_update": [],
                                          "on_wait": extra[gi:gi + _MAX_WAITS]},
                        })
                    si = dict(si)
                    si["on_wait"] = keep
                    inst = dict(inst)
                    inst["sync_info"] = si
                new_insts.append(inst)
            bb["instructions"] = new_insts
    return json.dumps(j).encode() if changed else bir_json


def _install_birfix():
    import concourse.bass_utils as bu
    if getattr(bu, "_birfix_installed", False):
        return
    orig = bu.compile_bir_kernel

    def patched(bir_json, tmpdir, neff_name="file.neff"):
        try:
            bir_json = _fix_bir_bytes(bir_json)
        except Exception as e:
            print("birfix failed:", e)
        return orig(bir_json, tmpdir, neff_name=neff_name)

    bu.compile_bir_kernel = patched
    try:
        import concourse.bass2jax as b2j
        b2j.compile_bir_kernel = patched
    except Exception as e:
        print("birfix bass2jax hook failed:", e)
    bu._birfix_installed = True


_install_birfix()


# ---------------------------------------------------------------------------
# run_bass_via_pjrt rebuilds jax.jit + recompiles the NEFF on EVERY call.
# Memoize the jitted sharded executable per Bass object so warm calls only
# pay input transfer + device execution.
def _install_pjrt_cache():
    import concourse.bass2jax as b2j

    if getattr(b2j, "_pjrt_cache_installed", False):
        return
    import jax
    from jax.sharding import Mesh, PartitionSpec
    from jax.experimental.shard_map import shard_map

    _runners = {}

    def _build_runner(nc, n_cores):
        b2j.install_neuronx_cc_hook()
        partition_name = (
            nc.partition_id_tensor.name if nc.partition_id_tensor else None
        )
        in_names, out_names, out_avals = [], [], []
        for alloc in nc.m.functions[0].allocations:
            if not isinstance(alloc, mybir.MemoryLocationSet):
                continue
            name = alloc.memorylocations[0].name
            if alloc.kind == "ExternalInput":
                if name != partition_name:
                    in_names.append(name)
            elif alloc.kind == "ExternalOutput":
                out_names.append(name)
                out_avals.append(
                    jax.core.ShapedArray(
                        tuple(alloc.tensor_shape), mybir.dt.np(alloc.dtype)
                    )
                )
        n_params = len(in_names)
        n_outs = len(out_avals)
        all_in = in_names + out_names + ([partition_name] if partition_name else [])
        donate = tuple(range(n_params, n_params + n_outs))

        def _body(*args):
            operands = list(args)
            if partition_name is not None:
                operands.append(b2j.partition_id_tensor())
            outs = b2j._bass_exec_p.bind(
                *operands,
                out_avals=tuple(out_avals),
                in_names=tuple(all_in),
                out_names=tuple(out_names),
                lowering_input_output_aliases=(),
                sim_require_finite=True,
                sim_require_nnan=True,
                nc=nc,
            )
            return tuple(outs)

        devices = jax.devices()[:n_cores]
        mesh = Mesh(np.asarray(devices), ("core",))
        spec = PartitionSpec("core")
        sharded = jax.jit(
            shard_map(
                _body,
                mesh=mesh,
                in_specs=(spec,) * (n_params + n_outs),
                out_specs=(spec,) * n_outs,
                check_rep=False,
            ),
            donate_argnums=donate,
            keep_unused=True,
        )
        # donated output buffers created on-device (no host->device zeros)
        import jax.numpy as jnp
        from jax.sharding import NamedSharding

        zshard = NamedSharding(mesh, spec)
        zfns = [
            jax.jit(
                (lambda shape, dtype: (lambda: jnp.zeros(shape, dtype)))(
                    (n_cores * av.shape[0], *av.shape[1:]), av.dtype
                ),
                out_shardings=zshard,
            )
            for av in out_avals
        ]

        b2j._pjrt_internals = {
            "sharded": sharded, "in_names": in_names, "out_names": out_names,
            "out_avals": out_avals, "mesh": mesh, "spec": spec, "zfns": zfns,
        }

        def run(in_maps=None, concat=None):
            if concat is not None:
                concat_in = [
                    np.ascontiguousarray(concat[name]) for name in in_names
                ]
            else:
                if nc.dbg_addr is not None:
                    in_maps = [
                        {**m, nc.dbg_addr.name: np.zeros((1, 2), np.uint32)}
                        for m in in_maps
                    ]
                per_core = [
                    [np.asarray(m[name]) for name in in_names] for m in in_maps
                ]
                concat_in = [
                    np.concatenate([per_core[c][i] for c in range(n_cores)], axis=0)
                    for i in range(n_params)
                ]
            concat_zeros = [zf() for zf in zfns]
            out_arrs = sharded(*concat_in, *concat_zeros)
            return [
                {
                    name: np.asarray(out_arrs[i]).reshape(
                        n_cores, *out_avals[i].shape
                    )[c]
                    for i, name in enumerate(out_names)
                }
                for c in range(n_cores)
            ]

        return run

    def cached(nc, in_maps, n_cores):
        key = (id(nc), n_cores)
        if key not in _runners:
            _runners[key] = _build_runner(nc, n_cores)
        if len(in_maps) == 1 and "__concat__" in in_maps[0]:
            return _runners[key](concat=in_maps[0]["__concat__"])
        return _runners[key](in_maps)

    b2j.run_bass_via_pjrt = cached
    b2j._pjrt_cache_installed = True


_install_pjrt_cache()

# ---------------------------------------------------------------------------
_CACHE = {}
_LAST_DEVICE_WALL = 0.0

NMT = 4   # row-tiles of 128 per core
NCC = 8   # column chunks of 512
NKC = 9   # K-tiles of 128 (1024 g-features + 4 augmented rows, padded)
KAUG = 1152  # 9 * 128


def _build_fused():
    nc = bass.Bass("TRN2", num_devices=NC)
    RK = nc.dram_tensor("RK", [KAUG, BLK], f16, kind="ExternalInput")
    LA = nc.dram_tensor("LA", [4, BLK], f16, kind="ExternalInput")
    NEB = nc.dram_tensor("NEB", [BLK, N], f16, kind="ExternalInput")
    EF = nc.dram_tensor("EF", [BLK, N], f16, kind="ExternalInput")
    Y1I = nc.dram_tensor("Y1I", [BLK, 1024], f16, kind="ExternalInput")
    WBK = nc.dram_tensor("WBK", [128, 512], f16, kind="ExternalInput")
    BR = nc.dram_tensor("BR", [1, 1536], f16, kind="ExternalInput")
    SM = nc.dram_tensor("SM", [128, 16], f32, kind="ExternalInput")
    ZOUT = nc.dram_tensor("ZOUT", [BLK, 512], f16, kind="ExternalOutput")

    identity_np = np.eye(128, dtype=np.float16)
    ones_col_np = np.ones((128, 1), dtype=np.float16)
    ones_row_np = np.ones((1, 128), dtype=np.float16)
    IDC = nc.inline_tensor(identity_np, name="idc")
    OCC = nc.inline_tensor(ones_col_np, name="occ")
    ORC = nc.inline_tensor(ones_row_np, name="orc")

    rg = [list(range(NC))]

    with TileContext(nc) as tc:
        with (
            tc.tile_pool(name="sb", bufs=1) as sb,
            tc.tile_pool(name="rfp", bufs=2) as rfp,
            tc.tile_pool(name="io", bufs=3) as io,
            tc.tile_pool(name="ps", bufs=4, space="PSUM") as ps,
            tc.tile_pool(name="dr", bufs=1, space="DRAM") as dr,
        ):
            # ---------------- DRAM scratch + collectives wiring ------------
            rgin = dr.tile([KAUG, BLK], f16)
            rgout = dr.tile([NC, KAUG, BLK], f16, addr_space="Shared")
            wbin = dr.tile([128, 512], f16)
            wbout = dr.tile([NC, 128, 512], f16, addr_space="Shared")
            degb = dr.tile([N], f32)
            degs = dr.tile([BLK], f32)
            pb = dr.tile([N, 1024], f16)
            rs1 = dr.tile([BLK, 1024], f16)
            p2b = dr.tile([N, 512], f16)
            rs2 = dr.tile([BLK, 512], f16)

            nc.sync.dma_start(rgin[:], RK[:])
            nc.gpsimd.collective_compute(
                "AllGather", OP.bypass, replica_groups=rg,
                ins=[rgin[:].opt()], outs=[rgout[:].opt()],
            )
            nc.sync.dma_start(wbin[:], WBK[:])
            nc.gpsimd.collective_compute(
                "AllGather", OP.bypass, replica_groups=rg,
                ins=[wbin[:].opt()], outs=[wbout[:].opt()],
            )

            # ---------------- constants / scalars --------------------------
            ident = sb.tile([128, 128], f16)
            nc.sync.dma_start(ident[:], IDC[:])
            onescol = sb.tile([128, 1], f16)
            nc.sync.dma_start(onescol[:], OCC[:])
            onesrow = sb.tile([1, 128], f16)
            nc.sync.dma_start(onesrow[:], ORC[:])
            sm = sb.tile([128, 16], f32)
            nc.sync.dma_start(sm[:], SM[:])
            brs = sb.tile([1, 1536], f16)
            nc.sync.dma_start(brs[:], BR[:])

            # bias broadcast via K=1 matmul: ones_col x bias_row
            b0bc = sb.tile([128, 1024], f32)
            b1bc = sb.tile([128, 512], f32)
            for j in range(3):
                psb = ps.tile([128, 512], f32, tag="ps", name=f"psb{j}")
                nc.tensor.matmul(psb[:], onesrow[:], brs[:, j * 512:(j + 1) * 512],
                                 start=True, stop=True)
                dst = b0bc[:, j * 512:(j + 1) * 512] if j < 2 else b1bc[:]
                nc.scalar.copy(dst, psb[:])

            # ---------------- lhsT for the gram stage ----------------------
            # lk[:, kc, :] = RK[kc*128:(kc+1)*128, :]; K-tile 8 swaps the
            # augmented rows: lhsT aug = [vhi; vlo; 1; 1] vs rhs [1; 1; vhi; vlo]
            lk = sb.tile([128, NKC, BLK], f16)
            for kc in range(8):
                nc.sync.dma_start(lk[:, kc, :], RK[kc * 128:(kc + 1) * 128, :])
            nc.vector.memset(lk[:, 8, :], 0.0)
            nc.sync.dma_start(lk[0:4, 8, :], LA[:])

            # ---------------- gram + ws = exp(2q) ---------------------------
            # wst holds ws (f16) per (mt, c) chunk; the elementwise chain
            # later overwrites each chunk in place with the final A values.
            wst = sb.tile([128, NMT, NCC, 512], f16)
            sqa = sb.tile([128, 32], f32)
            for c in range(NCC):
                rf = rfp.tile([128, NKC, 512], f16, tag="rf")
                for kc in range(NKC):
                    nc.sync.dma_start(
                        rf[:, kc, :], rgout[c, kc * 128:(kc + 1) * 128, :])
                for mt in range(NMT):
                    psg = ps.tile([128, 512], f32, tag="ps", name=f"psg{c}_{mt}")
                    for kc in range(NKC):
                        nc.tensor.matmul(
                            psg[:], lk[:, kc, mt * 128:(mt + 1) * 128],
                            rf[:, kc, :], start=(kc == 0), stop=(kc == NKC - 1))
                    wsl = wst[:, mt, c, :]
                    nc.scalar.activation(wsl, psg[:], AF.Exp, scale=2.0)
                    junk = io.tile([128, 512], f16, tag="junk", bufs=2)
                    nc.scalar.activation(
                        junk[:], wsl, AF.Square,
                        accum_out=sqa[:, mt * 8 + c:mt * 8 + c + 1])

            # ---------------- row norms + delta (self-loop indicator) ------
            rn4 = sb.tile([128, 4], f32)     # 1/||ws_row||
            rnb4 = sb.tile([128, 4], f32)    # (1-b) * rn4
            dl4 = sb.tile([128, 4], f32)     # delta in {0,1}
            for mt in range(NMT):
                rtmp = io.tile([128, 1], f32, tag="rtmp")
                nc.vector.tensor_reduce(
                    rtmp[:], sqa[:, mt * 8:(mt + 1) * 8],
                    axis=mybir.AxisListType.X, op=OP.add)
                rsq = io.tile([128, 1], f32, tag="rsq")
                nc.scalar.sqrt(rsq[:], rtmp[:])
                nc.vector.reciprocal(rn4[:, mt:mt + 1], rsq[:])
                nc.vector.tensor_scalar(
                    out=rnb4[:, mt:mt + 1], in0=rn4[:, mt:mt + 1],
                    scalar1=sm[:, 13:14], scalar2=None, op0=OP.mult)
                # delta chain on [128,1] diagonal scalars
                wsn = io.tile([128, 1], f32, tag="wsn")
                nc.vector.tensor_tensor(
                    out=wsn[:], in0=sm[:, mt:mt + 1], in1=rn4[:, mt:mt + 1],
                    op=OP.mult)
                t1d = io.tile([128, 1], f32, tag="t1d")
                nc.vector.scalar_tensor_tensor(
                    out=t1d[:], in0=wsn[:], scalar=sm[:, 13:14],
                    in1=sm[:, 4 + mt:5 + mt], op0=OP.mult, op1=OP.add)
                nc.vector.tensor_scalar(
                    out=t1d[:], in0=t1d[:], scalar1=1.0 - EPS_CLAMP,
                    scalar2=EPS_CLAMP, op0=OP.min, op1=OP.max)
                pd = io.tile([128, 1], f32, tag="pd")
                nc.vector.tensor_tensor(
                    out=pd[:], in0=t1d[:], in1=sm[:, 8 + mt:9 + mt], op=OP.mult)
                ndd = io.tile([128, 1], f32, tag="ndd")
                nc.vector.scalar_tensor_tensor(
                    out=ndd[:], in0=t1d[:], scalar=1.0, in1=pd[:],
                    op0=OP.subtract, op1=OP.subtract)
                rdd = io.tile([128, 1], f32, tag="rdd")
                nc.vector.reciprocal(rdd[:], ndd[:])
                qdd = io.tile([128, 1], f32, tag="qdd")
                nc.vector.tensor_tensor(
                    out=qdd[:], in0=pd[:], in1=rdd[:], op=OP.mult)
                nc.vector.tensor_scalar(
                    out=dl4[:, mt:mt + 1], in0=qdd[:], scalar1=sm[:, 12:13],
                    scalar2=None, op0=OP.is_ge)

            # ---------------- elementwise chain -> A (f16, in wst) ----------
            # term = clip((1-b)*ws_n + b*ne); A = sig(logit(term)+logit(e))
            # with sig(logit(t)+logit(e)) = t*E/(t*E + 1 - t),  E = e/(1-e)
            for c in range(NCC):
                for mt in range(NMT):
                    neb = io.tile([128, 512], f16, tag="neb")
                    nc.sync.dma_start(
                        neb[:], NEB[mt * 128:(mt + 1) * 128, c * 512:(c + 1) * 512])
                    eft = io.tile([128, 512], f16, tag="eft")
                    nc.sync.dma_start(
                        eft[:], EF[mt * 128:(mt + 1) * 128, c * 512:(c + 1) * 512])
                    efc = io.tile([128, 512], f32, tag="efc")
                    nc.scalar.copy(efc[:], eft[:])
                    t1 = io.tile([128, 512], f32, tag="t1")
                    nc.vector.scalar_tensor_tensor(
                        out=t1[:], in0=wst[:, mt, c, :], scalar=rnb4[:, mt:mt + 1],
                        in1=neb[:], op0=OP.mult, op1=OP.add)
                    nc.vector.tensor_scalar(
                        out=t1[:], in0=t1[:], scalar1=1.0 - EPS_CLAMP,
                        scalar2=EPS_CLAMP, op0=OP.min, op1=OP.max)
                    pt = io.tile([128, 512], f32, tag="pt")
                    nc.vector.tensor_tensor(
                        out=pt[:], in0=t1[:], in1=efc[:], op=OP.mult)
                    nd = io.tile([128, 512], f32, tag="efc")
                    nc.vector.scalar_tensor_tensor(
                        out=nd[:], in0=t1[:], scalar=1.0, in1=pt[:],
                        op0=OP.subtract, op1=OP.subtract)
                    rc = io.tile([128, 512], f32, tag="t1")
                    nc.vector.reciprocal(rc[:], nd[:])
                    q1 = io.tile([128, 512], f32, tag="pt")
                    nc.vector.tensor_tensor(
                        out=q1[:], in0=pt[:], in1=rc[:], op=OP.mult)
                    msk = io.tile([128, 512], f32, tag="msk")
                    nc.vector.tensor_scalar(
                        out=msk[:], in0=q1[:], scalar1=sm[:, 12:13],
                        scalar2=None, op0=OP.is_lt)
                    nc.vector.scalar_tensor_tensor(
                        out=wst[:, mt, c, :], in0=q1[:], scalar=-1.0,
                        in1=msk[:], op0=OP.mult, op1=OP.mult)

            # ---------------- column-degree partials -> RS ------------------
            for c in range(NCC):
                psd = ps.tile([1, 512], f32, tag="ps", name=f"psd{c}")
                for mt in range(NMT):
                    nc.tensor.matmul(psd[:], onescol[:], wst[:, mt, c, :],
                                     start=(mt == 0), stop=(mt == NMT - 1))
                degc = io.tile([1, 512], f32, tag="degc", bufs=2)
                nc.scalar.copy(degc[:], psd[:])
                nc.sync.dma_start(degb[c * 512:(c + 1) * 512], degc[:])
            nc.gpsimd.collective_compute(
                "ReduceScatter", OP.add, replica_groups=rg,
                ins=[degb[:].opt()], outs=[degs[:].opt()],
            )
            degl = sb.tile([128, 4], f32)
            nc.sync.dma_start(degl[:], degs.rearrange("(t p) -> p t", p=128))
            degf = sb.tile([128, 4], f32)
            nc.vector.tensor_tensor(out=degf[:], in0=degl[:], in1=dl4[:], op=OP.add)
            dsq = sb.tile([128, 4], f32)
            nc.scalar.sqrt(dsq[:], degf[:])
            dis4 = sb.tile([128, 4], f32)
            nc.vector.reciprocal(dis4[:], dsq[:])

            # ---------------- Y1 scaled ------------------------------------
            y1s = sb.tile([128, NMT, 1024], f16)
            for mt in range(NMT):
                nc.sync.dma_start(y1s[:, mt, :], Y1I[mt * 128:(mt + 1) * 128, :])
                nc.scalar.activation(y1s[:, mt, :], y1s[:, mt, :], AF.Copy,
                                     scale=dis4[:, mt:mt + 1])

            # ---------------- layer-1 aggregation partials -> RS ------------
            for c in range(NCC):
                for w in range(4):
                    psa = ps.tile([128, 1024], f32, tag="ps", name=f"psa{c}_{w}")
                    for mt in range(NMT):
                        lhsT = wst[:, mt, c, w * 128:(w + 1) * 128]
                        for nn in range(2):
                            nc.tensor.matmul(
                                psa[:, nn * 512:(nn + 1) * 512], lhsT,
                                y1s[:, mt, nn * 512:(nn + 1) * 512],
                                start=(mt == 0), stop=(mt == NMT - 1))
                    o1 = io.tile([128, 1024], f16, tag="o1")
                    nc.scalar.copy(o1[:], psa[:])
                    nc.sync.dma_start(
                        pb[c * 512 + w * 128:c * 512 + (w + 1) * 128, :], o1[:])
            nc.gpsimd.collective_compute(
                "ReduceScatter", OP.add, replica_groups=rg,
                ins=[pb[:].opt()], outs=[rs1[:].opt()],
            )

            # ---------------- layer-1 post: h = relu(dis*(S + delta*Y1s) + b0)
            h16 = sb.tile([128, NMT, 1024], f16)
            for mt in range(NMT):
                rst = io.tile([128, 1024], f16, tag="rst", bufs=2)
                nc.sync.dma_start(rst[:], rs1[mt * 128:(mt + 1) * 128, :])
                s1 = io.tile([128, 1024], f32, tag="s1", bufs=2)
                nc.vector.scalar_tensor_tensor(
                    out=s1[:], in0=y1s[:, mt, :], scalar=dl4[:, mt:mt + 1],
                    in1=rst[:], op0=OP.mult, op1=OP.add)
                nc.vector.scalar_tensor_tensor(
                    out=s1[:], in0=s1[:], scalar=dis4[:, mt:mt + 1],
                    in1=b0bc[:], op0=OP.mult, op1=OP.add)
                nc.scalar.activation(h16[:, mt, :], s1[:], AF.Relu)

            # ---------------- Y2 = (h @ blockdiag(mW1,sW1)) * dis -----------
            wbt = sb.tile([128, 8, 512], f16)
            for kb in range(8):
                nc.sync.dma_start(wbt[:, kb, :], wbout[kb, :, :])
            y2s = sb.tile([128, NMT, 512], f16)
            for mt in range(NMT):
                htb = io.tile([128, 8, 128], f16, tag="htb", bufs=2)
                for kb in range(8):
                    pst = ps.tile([128, 128], f16, tag="ps", name=f"pst{mt}_{kb}")
                    nc.tensor.transpose(
                        pst[:], h16[:, mt, kb * 128:(kb + 1) * 128], ident[:])
                    nc.scalar.copy(htb[:, kb, :], pst[:])
                psy = ps.tile([128, 512], f32, tag="ps", name=f"psy{mt}")
                for kb in range(8):
                    nc.tensor.matmul(psy[:], htb[:, kb, :], wbt[:, kb, :],
                                     start=(kb == 0), stop=(kb == 7))
                nc.scalar.activation(y2s[:, mt, :], psy[:], AF.Copy,
                                     scale=dis4[:, mt:mt + 1])

            # ---------------- layer-2 aggregation partials -> RS ------------
            for c in range(NCC):
                for w in range(4):
                    ps2 = ps.tile([128, 512], f32, tag="ps", name=f"ps2{c}_{w}")
                    for mt in range(NMT):
                        nc.tensor.matmul(
                            ps2[:], wst[:, mt, c, w * 128:(w + 1) * 128],
                            y2s[:, mt, :], start=(mt == 0), stop=(mt == NMT - 1))
                    o2 = io.tile([128, 512], f16, tag="o2")
                    nc.scalar.copy(o2[:], ps2[:])
                    nc.sync.dma_start(
                        p2b[c * 512 + w * 128:c * 512 + (w + 1) * 128, :], o2[:])
            nc.gpsimd.collective_compute(
                "ReduceScatter", OP.add, replica_groups=rg,
                ins=[p2b[:].opt()], outs=[rs2[:].opt()],
            )

            # ---------------- layer-2 post + output -------------------------
            for mt in range(NMT):
                r2t = io.tile([128, 512], f16, tag="r2t", bufs=2)
                nc.sync.dma_start(r2t[:], rs2[mt * 128:(mt + 1) * 128, :])
                z1 = io.tile([128, 512], f32, tag="z1", bufs=2)
                nc.vector.scalar_tensor_tensor(
                    out=z1[:], in0=y2s[:, mt, :], scalar=dl4[:, mt:mt + 1],
                    in1=r2t[:], op0=OP.mult, op1=OP.add)
                nc.vector.scalar_tensor_tensor(
                    out=z1[:], in0=z1[:], scalar=dis4[:, mt:mt + 1],
                    in1=b1bc[:], op0=OP.mult, op1=OP.add)
                zo = io.tile([128, 512], f16, tag="zo", bufs=2)
                nc.scalar.activation(zo[:], z1[:], AF.Relu)
                nc.sync.dma_start(ZOUT[mt * 128:(mt + 1) * 128, :], zo[:])
    return nc


def _run(name, builder, in_maps):
    global _LAST_DEVICE_WALL
    if name not in _CACHE:
        _CACHE[name] = builder()
    t0 = time.time()
    res = run_bass_kernel_spmd(_CACHE[name], in_maps, core_ids=list(range(NC)))
    dt = time.time() - t0
    _LAST_DEVICE_WALL += dt
    import os
    if os.environ.get("KERNEL_DEBUG_TIMES"):
        print(f"  [dev call {name}: {dt * 1e3:.1f} ms]")
    return res.results


# ---------------------------------------------------------------------------
def _l2n(x):
    n = np.sqrt(np.sum(x * x, axis=1, keepdims=True))
    return x / np.maximum(n, 1e-12)


def kernel(x, new_edge, beta, delta, eps, Wm, bm, Ws, bs,
           mW0, mb0, mW1, mb1, sW0, sb0, sW1, sb1):
    global _LAST_DEVICE_WALL
    _LAST_DEVICE_WALL = 0.0
    x = np.asarray(x, np.float32)
    b = float(np.asarray(beta).reshape(-1)[0])
    d = float(np.asarray(delta).reshape(-1)[0])

    x_mean = x @ Wm + bm
    x_std = x @ Ws + bs

    m = _l2n(x_mean)
    c = _l2n(np.exp(x_std))
    cs = np.sqrt(c)
    sq = np.sum(m * m, axis=1)
    csum = np.sum(c, axis=1)
    u = (sq + csum).astype(np.float32)

    g = np.concatenate([m, cs], axis=1).astype(np.float32)  # [N, 1024]
    v = -0.5 * u
    vhi = v.astype(np.float16)
    vlo = (v - vhi.astype(np.float32)).astype(np.float16)

    # R (rhs side of the gram): rows 0:1024 g.T; 1024-25: ones; 1026-27: v
    R = np.zeros((KAUG, N), np.float16)
    R[0:1024] = g.T.astype(np.float16)
    R[1024] = 1.0
    R[1025] = 1.0
    R[1026] = vhi
    R[1027] = vlo

    ne = np.asarray(new_edge, np.float32)
    NEBf = (b * ne).astype(np.float16)
    e = np.clip(np.asarray(eps, np.float32), EPS_CLAMP, 1.0 - EPS_CLAMP)
    Ef = np.minimum(e / (1.0 - e), 60000.0).astype(np.float16)

    Y1 = np.concatenate([x_mean @ mW0, x_std @ sW0], axis=1).astype(np.float16)

    WBD = np.zeros((1024, 512), np.float16)
    WBD[:512, :256] = mW1
    WBD[512:, 256:] = sW1

    BRv = np.concatenate([mb0, sb0, mb1, sb1]).astype(np.float16).reshape(1, 1536)

    # host diagonal scalars (match the on-device dataflow, f32 approx)
    g2 = np.sum(g * g, axis=1)
    wsdiag = np.exp(-(2.0 * u - 2.0 * g2)).astype(np.float32)
    nebdiag = np.diagonal(NEBf).astype(np.float32)
    Ediag = np.diagonal(Ef).astype(np.float32)

    # pre-concatenated (all-cores) inputs: axis0 = core-major, zero extra
    # copies for the row-sharded tensors
    RK_cat = np.ascontiguousarray(
        R.reshape(KAUG, NC, BLK).transpose(1, 0, 2).reshape(NC * KAUG, BLK))
    LA_cat = np.empty((NC, 4, BLK), np.float16)
    LA_cat[:, 0, :] = vhi.reshape(NC, BLK)
    LA_cat[:, 1, :] = vlo.reshape(NC, BLK)
    LA_cat[:, 2:4, :] = 1.0
    LA_cat = LA_cat.reshape(NC * 4, BLK)
    SM_cat = np.zeros((NC, 128, 16), np.float32)
    SM_cat[:, :, 0:4] = wsdiag.reshape(NC, 4, 128).transpose(0, 2, 1)
    SM_cat[:, :, 4:8] = nebdiag.reshape(NC, 4, 128).transpose(0, 2, 1)
    SM_cat[:, :, 8:12] = Ediag.reshape(NC, 4, 128).transpose(0, 2, 1)
    SM_cat[:, :, 12] = -d
    SM_cat[:, :, 13] = 1.0 - b
    SM_cat = SM_cat.reshape(NC * 128, 16)
    concat_map = {
        "RK": RK_cat,
        "LA": LA_cat,
        "NEB": NEBf,
        "EF": Ef,
        "Y1I": Y1,
        "WBK": WBD,
        "BR": np.repeat(BRv, NC, axis=0),
        "SM": SM_cat,
    }

    res = _run("fused", _build_fused, [{"__concat__": concat_map}])
    z = np.concatenate([res[k]["ZOUT"] for k in range(NC)], axis=0)
    z_mean = np.ascontiguousarray(z[:, :H]).astype(np.float32)
    z_std = np.ascontiguousarray(z[:, H:2 * H]).astype(np.float32)
    return z_mean, z_std


# revision 33
# speedup vs baseline: 8.4241x; 1.1026x over previous
"""Trainium2 kernel for nn_GaussianModel (gnn_message_passing).

Single fused device call, row-sharded across 8 cores (core k owns rows
r_k = [k*512, (k+1)*512) of the 4096-node graph):

  host:   x_mean/x_std projections, l2-norms, Y1 = [xm@mW0 | xs@sW0],
          E = e/(1-e), b*new_edge  (cheap O(N*F) / elementwise prep)
  device: Gram via augmented-K matmul  q_ij = g_i.g_j - (u_i+u_j)/2
          ws = exp(2q), row-l2-norm, logit-mix with new_edge/eps
          (algebraic sigmoid form, no log/exp), threshold -> A row-block
          column degree partials -> ReduceScatter -> dis = deg^-1/2
          layer-1 aggregation partials A_k^T @ (dis*Y1) -> ReduceScatter
          relu(+b0), on-device h @ blockdiag(mW1,sW1) via PE transpose
          layer-2 aggregation partials -> ReduceScatter -> relu(+b1)
  The GCN self-loop (add 1 where diag==0) is handled analytically: a
  per-row delta in {0,1} computed from host-supplied diagonal scalars,
  added to deg and applied as a rank-1 correction after each RS.

All per-core variation lives in the input data (no partition-id control
flow); collectives (2x AllGather, 3x ReduceScatter) handle placement.
"""
import json
import sys
import time

sys.path.insert(0, "/opt/trn_rl_repo")
import numpy as np
import concourse.bass as bass
import concourse.mybir as mybir
from concourse.tile import TileContext
from concourse.bass_utils import run_bass_kernel_spmd

NC = 8
N, F, H = 4096, 512, 256
BLK = N // NC  # 512 rows per core
EPS_CLAMP = 1e-6
f32, f16 = mybir.dt.float32, mybir.dt.float16
AF = mybir.ActivationFunctionType
OP = mybir.AluOpType

# ---------------------------------------------------------------------------
# walrus in this container caps sem-waits at 1 per instruction; Tile emits
# more. Split excess waits onto preceding same-engine Drains in the BIR JSON.
_MAX_WAITS = 1


def _fix_bir_bytes(bir_json):
    j = json.loads(bir_json)
    changed = False
    for fn in j.get("functions", []):
        for bb in fn.get("blocks", []):
            new_insts = []
            for inst in bb.get("instructions", []):
                si = inst.get("sync_info") or {}
                waits = si.get("on_wait") or []
                if len(waits) > _MAX_WAITS and inst.get("engine", "Unassigned") != "Unassigned":
                    changed = True
                    keep = waits[-_MAX_WAITS:]
                    extra = waits[:-_MAX_WAITS]
                    for gi in range(0, len(extra), _MAX_WAITS):
                        new_insts.append({
                            "debug": inst.get("debug", 0),
                            "engine": inst["engine"],
                            "ins": [],
                            "outs": [],
                            "name": f"{inst['name']}-ws{gi}",
                            "opcode": "Drain",
                            "sync_info": {"on_update": [],
                                          "on_wait": extra[gi:gi + _MAX_WAITS]},
                        })
                    si = dict(si)
                    si["on_wait"] = keep
                    inst = dict(inst)
                    inst["sync_info"] = si
                new_insts.append(inst)
            bb["instructions"] = new_insts
    return json.dumps(j).encode() if changed else bir_json


def _install_birfix():
    import concourse.bass_utils as bu
    if getattr(bu, "_birfix_installed", False):
        return
    orig = bu.compile_bir_kernel

    def patched(bir_json, tmpdir, neff_name="file.neff"):
        try:
            bir_json = _fix_bir_bytes(bir_json)
        except Exception as e:
            print("birfix failed:", e)
        return orig(bir_json, tmpdir, neff_name=neff_name)

    bu.compile_bir_kernel = patched
    try:
        import concourse.bass2jax as b2j
        b2j.compile_bir_kernel = patched
    except Exception as e:
        print("birfix bass2jax hook failed:", e)
    bu._birfix_installed = True


_install_birfix()


# ---------------------------------------------------------------------------
# run_bass_via_pjrt rebuilds jax.jit + recompiles the NEFF on EVERY call.
# Memoize the jitted sharded executable per Bass object so warm calls only
# pay input transfer + device execution.
def _install_pjrt_cache():
    import concourse.bass2jax as b2j

    if getattr(b2j, "_pjrt_cache_installed", False):
        return
    import jax
    from jax.sharding import Mesh, PartitionSpec
    from jax.experimental.shard_map import shard_map

    _runners = {}

    def _build_runner(nc, n_cores):
        b2j.install_neuronx_cc_hook()
        partition_name = (
            nc.partition_id_tensor.name if nc.partition_id_tensor else None
        )
        in_names, out_names, out_avals = [], [], []
        for alloc in nc.m.functions[0].allocations:
            if not isinstance(alloc, mybir.MemoryLocationSet):
                continue
            name = alloc.memorylocations[0].name
            if alloc.kind == "ExternalInput":
                if name != partition_name:
                    in_names.append(name)
            elif alloc.kind == "ExternalOutput":
                out_names.append(name)
                out_avals.append(
                    jax.core.ShapedArray(
                        tuple(alloc.tensor_shape), mybir.dt.np(alloc.dtype)
                    )
                )
        n_params = len(in_names)
        n_outs = len(out_avals)
        all_in = in_names + out_names + ([partition_name] if partition_name else [])
        donate = tuple(range(n_params, n_params + n_outs))

        def _body(*args):
            operands = list(args)
            if partition_name is not None:
                operands.append(b2j.partition_id_tensor())
            outs = b2j._bass_exec_p.bind(
                *operands,
                out_avals=tuple(out_avals),
                in_names=tuple(all_in),
                out_names=tuple(out_names),
                lowering_input_output_aliases=(),
                sim_require_finite=True,
                sim_require_nnan=True,
                nc=nc,
            )
            return tuple(outs)

        devices = jax.devices()[:n_cores]
        mesh = Mesh(np.asarray(devices), ("core",))
        spec = PartitionSpec("core")
        sharded = jax.jit(
            shard_map(
                _body,
                mesh=mesh,
                in_specs=(spec,) * (n_params + n_outs),
                out_specs=(spec,) * n_outs,
                check_rep=False,
            ),
            donate_argnums=donate,
            keep_unused=True,
        )
        # donated output buffers created on-device (no host->device zeros)
        import jax.numpy as jnp
        from jax.sharding import NamedSharding

        zshard = NamedSharding(mesh, spec)
        zfns = [
            jax.jit(
                (lambda shape, dtype: (lambda: jnp.zeros(shape, dtype)))(
                    (n_cores * av.shape[0], *av.shape[1:]), av.dtype
                ),
                out_shardings=zshard,
            )
            for av in out_avals
        ]

        b2j._pjrt_internals = {
            "sharded": sharded, "in_names": in_names, "out_names": out_names,
            "out_avals": out_avals, "mesh": mesh, "spec": spec, "zfns": zfns,
        }

        def run(in_maps=None, concat=None):
            if concat is not None:
                concat_in = [
                    np.ascontiguousarray(concat[name]) for name in in_names
                ]
            else:
                if nc.dbg_addr is not None:
                    in_maps = [
                        {**m, nc.dbg_addr.name: np.zeros((1, 2), np.uint32)}
                        for m in in_maps
                    ]
                per_core = [
                    [np.asarray(m[name]) for name in in_names] for m in in_maps
                ]
                concat_in = [
                    np.concatenate([per_core[c][i] for c in range(n_cores)], axis=0)
                    for i in range(n_params)
                ]
            concat_zeros = [zf() for zf in zfns]
            out_arrs = sharded(*concat_in, *concat_zeros)
            return [
                {
                    name: np.asarray(out_arrs[i]).reshape(
                        n_cores, *out_avals[i].shape
                    )[c]
                    for i, name in enumerate(out_names)
                }
                for c in range(n_cores)
            ]

        return run

    def cached(nc, in_maps, n_cores):
        key = (id(nc), n_cores)
        if key not in _runners:
            _runners[key] = _build_runner(nc, n_cores)
        if len(in_maps) == 1 and "__concat__" in in_maps[0]:
            return _runners[key](concat=in_maps[0]["__concat__"])
        return _runners[key](in_maps)

    b2j.run_bass_via_pjrt = cached
    b2j._pjrt_cache_installed = True


_install_pjrt_cache()

# ---------------------------------------------------------------------------
_CACHE = {}
_LAST_DEVICE_WALL = 0.0

NMT = 4   # row-tiles of 128 per core
NCC = 8   # column chunks of 512
NKC = 9   # K-tiles of 128 (1024 g-features + 4 augmented rows, padded)
KAUG = 1152  # 9 * 128


def _build_fused():
    nc = bass.Bass("TRN2", num_devices=NC)
    # x.T column-slice for this core's rows: [infeat 512, nodes 512]
    XI = nc.dram_tensor("XI", [F, BLK], f16, kind="ExternalInput")
    # [Wm | Ws | Wm@mW0 | Ws@sW0] row-shard: [64, 2048] (AllGathered)
    WCK = nc.dram_tensor("WCK", [F // NC, 4 * F], f16, kind="ExternalInput")
    # bias rows: [bm | bs | bm@mW0 | bs@sW0], [1, 2048]
    BR2 = nc.dram_tensor("BR2", [1, 4 * F], f32, kind="ExternalInput")
    NEB = nc.dram_tensor("NEB", [BLK, N], f16, kind="ExternalInput")
    EF = nc.dram_tensor("EF", [BLK, N], f16, kind="ExternalInput")
    WBK = nc.dram_tensor("WBK", [128, 512], f16, kind="ExternalInput")
    BR = nc.dram_tensor("BR", [1, 1536], f16, kind="ExternalInput")
    SM = nc.dram_tensor("SM", [128, 16], f32, kind="ExternalInput")
    ZOUT = nc.dram_tensor("ZOUT", [BLK, 512], f16, kind="ExternalOutput")

    identity_np = np.eye(128, dtype=np.float16)
    ones_col_np = np.ones((128, 1), dtype=np.float16)
    ones_row_np = np.ones((1, 128), dtype=np.float16)
    augones_np = np.zeros((4, BLK), dtype=np.float16)
    augones_np[2:4] = 1.0
    IDC = nc.inline_tensor(identity_np, name="idc")
    OCC = nc.inline_tensor(ones_col_np, name="occ")
    ORC = nc.inline_tensor(ones_row_np, name="orc")
    AUGO = nc.inline_tensor(augones_np, name="augo")

    rg = [list(range(NC))]

    with TileContext(nc) as tc:
        with (
            tc.tile_pool(name="sb", bufs=1) as sb,
            tc.tile_pool(name="rfp", bufs=2) as rfp,
            tc.tile_pool(name="io", bufs=3) as io,
            tc.tile_pool(name="ps", bufs=4, space="PSUM") as ps,
            tc.tile_pool(name="dr", bufs=1, space="DRAM") as dr,
        ):
            # ---------------- DRAM scratch + collectives wiring ------------
            rgin = dr.tile([KAUG, BLK], f16)
            rgout = dr.tile([NC, KAUG, BLK], f16, addr_space="Shared")
            wcin = dr.tile([F // NC, 4 * F], f16)
            wcout = dr.tile([NC, F // NC, 4 * F], f16, addr_space="Shared")
            wbin = dr.tile([128, 512], f16)
            wbout = dr.tile([NC, 128, 512], f16, addr_space="Shared")
            degb = dr.tile([N], f32)
            degs = dr.tile([BLK], f32)
            pb = dr.tile([N, 1024], f16)
            rs1 = dr.tile([BLK, 1024], f16)
            p2b = dr.tile([N, 512], f16)
            rs2 = dr.tile([BLK, 512], f16)

            nc.sync.dma_start(wcin[:], WCK[:])
            nc.gpsimd.collective_compute(
                "AllGather", OP.bypass, replica_groups=rg,
                ins=[wcin[:].opt()], outs=[wcout[:].opt()],
            )
            nc.sync.dma_start(wbin[:], WBK[:])
            nc.gpsimd.collective_compute(
                "AllGather", OP.bypass, replica_groups=rg,
                ins=[wbin[:].opt()], outs=[wbout[:].opt()],
            )

            # ---------------- constants / scalars --------------------------
            ident = sb.tile([128, 128], f16)
            nc.sync.dma_start(ident[:], IDC[:])
            onescol = sb.tile([128, 1], f16)
            nc.sync.dma_start(onescol[:], OCC[:])
            onesrow = sb.tile([1, 128], f16)
            nc.sync.dma_start(onesrow[:], ORC[:])
            sm = sb.tile([128, 16], f32)
            nc.sync.dma_start(sm[:], SM[:])
            brs = sb.tile([1, 1536], f16)
            nc.sync.dma_start(brs[:], BR[:])
            # BR2 arrives f32; load then narrow to f16 for the K=1 matmuls
            br2f = sb.tile([1, 4 * F], f32)
            nc.sync.dma_start(br2f[:], BR2[:])
            br2h = sb.tile([1, 4 * F], f16)
            nc.scalar.copy(br2h[:], br2f[:])

            # bias broadcast via K=1 matmul: ones_col x bias_row
            b0bc = sb.tile([128, 1024], f32)
            b1bc = sb.tile([128, 512], f32)
            for j in range(3):
                psb = ps.tile([128, 512], f32, tag="ps", name=f"psb{j}")
                nc.tensor.matmul(psb[:], onesrow[:], brs[:, j * 512:(j + 1) * 512],
                                 start=True, stop=True)
                dst = b0bc[:, j * 512:(j + 1) * 512] if j < 2 else b1bc[:]
                nc.scalar.copy(dst, psb[:])

            # ---------------- x-projections, norms, g assembly -------------
            # lk[:, kc<8, :] = g.T feature-tiles (m kc 0..3, cs kc 4..7),
            # built on device from XI and the AllGathered weight blocks.
            xi = sb.tile([128, 4, BLK], f16)
            for kt in range(4):
                nc.sync.dma_start(xi[:, kt, :], XI[kt * 128:(kt + 1) * 128, :])
            wcflat = wcout.rearrange("a b c -> (a b) c")
            lk = sb.tile([128, NKC, BLK], f16)
            u4 = sb.tile([128, 4], f32)
            y1s = sb.tile([128, NMT, 1024], f16)
            for wblk in range(4):
                wcb = io.tile([128, 4, 512], f16, tag="wcb", bufs=2)
                for kt in range(4):
                    nc.sync.dma_start(
                        wcb[:, kt, :],
                        wcflat[kt * 128:(kt + 1) * 128,
                               wblk * 512:(wblk + 1) * 512])
                for mt in range(NMT):
                    pm = ps.tile([128, 512], f32, tag="ps", name=f"pm{wblk}_{mt}")
                    for kt in range(4):
                        nc.tensor.matmul(
                            pm[:], xi[:, kt, mt * 128:(mt + 1) * 128],
                            wcb[:, kt, :], start=(kt == 0), stop=False)
                    nc.tensor.matmul(
                        pm[:], onesrow[:],
                        br2h[:, wblk * 512:(wblk + 1) * 512],
                        start=False, stop=True)
                    if wblk == 0:
                        # m = l2-normalized x_mean, transposed into lk
                        xmn = io.tile([128, 512], f32, tag="xmn", bufs=2)
                        nc.scalar.copy(xmn[:], pm[:])
                        junk2 = io.tile([128, 512], f16, tag="junk", bufs=2)
                        sqx = io.tile([128, 1], f32, tag="sqx")
                        nc.scalar.activation(junk2[:], xmn[:], AF.Square,
                                             accum_out=sqx[:])
                        srx = io.tile([128, 1], f32, tag="srx")
                        nc.scalar.sqrt(srx[:], sqx[:])
                        rnx = io.tile([128, 1], f32, tag="rnx")
                        nc.vector.reciprocal(rnx[:], srx[:])
                        mtile = io.tile([128, 512], f16, tag="mtile", bufs=2)
                        nc.scalar.activation(mtile[:], xmn[:], AF.Copy,
                                             scale=rnx[:])
                        for ftb in range(4):
                            ptr = ps.tile([128, 128], f16, tag="ps",
                                          name=f"ptm{mt}_{ftb}")
                            nc.tensor.transpose(
                                ptr[:], mtile[:, ftb * 128:(ftb + 1) * 128],
                                ident[:])
                            nc.scalar.copy(
                                lk[:, ftb, mt * 128:(mt + 1) * 128], ptr[:])
                    elif wblk == 1:
                        # c = l2n(exp(x_std)); cs = sqrt(c); u = 1 + sum(c)
                        c0 = io.tile([128, 512], f32, tag="xmn", bufs=2)
                        cs0 = io.tile([128, 1], f32, tag="cs0")
                        nc.scalar.activation(c0[:], pm[:], AF.Exp,
                                             accum_out=cs0[:])
                        junk2 = io.tile([128, 512], f16, tag="junk", bufs=2)
                        sqx = io.tile([128, 1], f32, tag="sqx")
                        nc.scalar.activation(junk2[:], c0[:], AF.Square,
                                             accum_out=sqx[:])
                        srx = io.tile([128, 1], f32, tag="srx")
                        nc.scalar.sqrt(srx[:], sqx[:])
                        rnx = io.tile([128, 1], f32, tag="rnx")
                        nc.vector.reciprocal(rnx[:], srx[:])
                        cstile = io.tile([128, 512], f16, tag="mtile", bufs=2)
                        nc.scalar.activation(cstile[:], c0[:], AF.Sqrt,
                                             scale=rnx[:])
                        for ftb in range(4):
                            ptr = ps.tile([128, 128], f16, tag="ps",
                                          name=f"ptc{mt}_{ftb}")
                            nc.tensor.transpose(
                                ptr[:], cstile[:, ftb * 128:(ftb + 1) * 128],
                                ident[:])
                            nc.scalar.copy(
                                lk[:, 4 + ftb, mt * 128:(mt + 1) * 128],
                                ptr[:])
                        nc.vector.tensor_tensor(
                            out=u4[:, mt:mt + 1], in0=cs0[:], in1=rnx[:],
                            op=OP.mult)
                    else:
                        # Y1 halves (x @ (W @ W0) + fold(b)), unscaled yet
                        half = wblk - 2
                        nc.scalar.copy(
                            y1s[:, mt, half * 512:(half + 1) * 512], pm[:])

            # u = 1 + csum; v = -u/2 split into f16 hi/lo
            nc.vector.tensor_scalar(out=u4[:], in0=u4[:], scalar1=1.0,
                                    scalar2=None, op0=OP.add)
            vneg = sb.tile([128, 4], f32)
            nc.vector.tensor_scalar(out=vneg[:], in0=u4[:], scalar1=-0.5,
                                    scalar2=None, op0=OP.mult)
            vh4 = sb.tile([128, 4], f16)
            nc.scalar.copy(vh4[:], vneg[:])
            vh4f = sb.tile([128, 4], f32)
            nc.scalar.copy(vh4f[:], vh4[:])
            vl4 = sb.tile([128, 4], f16)
            nc.vector.tensor_tensor(out=vl4[:], in0=vneg[:], in1=vh4f[:],
                                    op=OP.subtract)

            # lk K-tile 8 (lhsT aug rows [vhi; vlo; 1; 1]) + rgin assembly
            nc.vector.memset(lk[:, 8, :], 0.0)
            nc.sync.dma_start(lk[0:4, 8, :], AUGO[:])
            pag = ps.tile([2, 512], f16, tag="ps", name="pag")
            for t in range(4):
                vhl = io.tile([128, 2], f16, tag="vhl", bufs=2)
                nc.vector.tensor_copy(vhl[:, 0:1], vh4[:, t:t + 1])
                nc.vector.tensor_copy(vhl[:, 1:2], vl4[:, t:t + 1])
                nc.tensor.transpose(pag[0:2, t * 128:(t + 1) * 128],
                                    vhl[:], ident[:])
            nc.scalar.copy(lk[0:2, 8, :], pag[:])

            # rgin (rhs side, aug rows swapped: [1; 1; vhi; vlo]) -> AllGather
            for kc in range(8):
                nc.sync.dma_start(rgin[kc * 128:(kc + 1) * 128, :], lk[:, kc, :])
            onetile = sb.tile([2, BLK], f16)
            nc.vector.memset(onetile[:], 1.0)
            nc.sync.dma_start(rgin[1024:1026, :], onetile[:])
            nc.sync.dma_start(rgin[1026:1028, :], lk[0:2, 8, :])
            ztile = sb.tile([124, BLK], f16)
            nc.vector.memset(ztile[:], 0.0)
            nc.sync.dma_start(rgin[1028:1152, :], ztile[:])
            nc.gpsimd.collective_compute(
                "AllGather", OP.bypass, replica_groups=rg,
                ins=[rgin[:].opt()], outs=[rgout[:].opt()],
            )

            # ---------------- gram + ws = exp(2q) ---------------------------
            # wst holds ws (f16) per (mt, c) chunk; the elementwise chain
            # later overwrites each chunk in place with the final A values.
            wst = sb.tile([128, NMT, NCC, 512], f16)
            sqa = sb.tile([128, 32], f32)
            for c in range(NCC):
                rf = rfp.tile([128, NKC, 512], f16, tag="rf")
                for kc in range(NKC):
                    nc.sync.dma_start(
                        rf[:, kc, :], rgout[c, kc * 128:(kc + 1) * 128, :])
                for mt in range(NMT):
                    psg = ps.tile([128, 512], f32, tag="ps", name=f"psg{c}_{mt}")
                    for kc in range(NKC):
                        nc.tensor.matmul(
                            psg[:], lk[:, kc, mt * 128:(mt + 1) * 128],
                            rf[:, kc, :], start=(kc == 0), stop=(kc == NKC - 1))
                    wsl = wst[:, mt, c, :]
                    nc.scalar.activation(wsl, psg[:], AF.Exp, scale=2.0)
                    junk = io.tile([128, 512], f16, tag="junk", bufs=2)
                    nc.scalar.activation(
                        junk[:], wsl, AF.Square,
                        accum_out=sqa[:, mt * 8 + c:mt * 8 + c + 1])

            # ---------------- row norms + delta (self-loop indicator) ------
            rn4 = sb.tile([128, 4], f32)     # 1/||ws_row||
            rnb4 = sb.tile([128, 4], f32)    # (1-b) * rn4
            dl4 = sb.tile([128, 4], f32)     # delta in {0,1}
            for mt in range(NMT):
                rtmp = io.tile([128, 1], f32, tag="rtmp")
                nc.vector.tensor_reduce(
                    rtmp[:], sqa[:, mt * 8:(mt + 1) * 8],
                    axis=mybir.AxisListType.X, op=OP.add)
                rsq = io.tile([128, 1], f32, tag="rsq")
                nc.scalar.sqrt(rsq[:], rtmp[:])
                nc.vector.reciprocal(rn4[:, mt:mt + 1], rsq[:])
                nc.vector.tensor_scalar(
                    out=rnb4[:, mt:mt + 1], in0=rn4[:, mt:mt + 1],
                    scalar1=sm[:, 13:14], scalar2=None, op0=OP.mult)
                # delta chain on [128,1] diagonal scalars
                wsn = io.tile([128, 1], f32, tag="wsn")
                nc.vector.tensor_tensor(
                    out=wsn[:], in0=sm[:, mt:mt + 1], in1=rn4[:, mt:mt + 1],
                    op=OP.mult)
                t1d = io.tile([128, 1], f32, tag="t1d")
                nc.vector.scalar_tensor_tensor(
                    out=t1d[:], in0=wsn[:], scalar=sm[:, 13:14],
                    in1=sm[:, 4 + mt:5 + mt], op0=OP.mult, op1=OP.add)
                nc.vector.tensor_scalar(
                    out=t1d[:], in0=t1d[:], scalar1=1.0 - EPS_CLAMP,
                    scalar2=EPS_CLAMP, op0=OP.min, op1=OP.max)
                pd = io.tile([128, 1], f32, tag="pd")
                nc.vector.tensor_tensor(
                    out=pd[:], in0=t1d[:], in1=sm[:, 8 + mt:9 + mt], op=OP.mult)
                ndd = io.tile([128, 1], f32, tag="ndd")
                nc.vector.scalar_tensor_tensor(
                    out=ndd[:], in0=t1d[:], scalar=1.0, in1=pd[:],
                    op0=OP.subtract, op1=OP.subtract)
                rdd = io.tile([128, 1], f32, tag="rdd")
                nc.vector.reciprocal(rdd[:], ndd[:])
                qdd = io.tile([128, 1], f32, tag="qdd")
                nc.vector.tensor_tensor(
                    out=qdd[:], in0=pd[:], in1=rdd[:], op=OP.mult)
                nc.vector.tensor_scalar(
                    out=dl4[:, mt:mt + 1], in0=qdd[:], scalar1=sm[:, 12:13],
                    scalar2=None, op0=OP.is_ge)

            # ---------------- elementwise chain -> A (f16, in wst) ----------
            # term = clip((1-b)*ws_n + b*ne); A = sig(logit(term)+logit(e))
            # with sig(logit(t)+logit(e)) = t*E/(t*E + 1 - t),  E = e/(1-e)
            for c in range(NCC):
                for mt in range(NMT):
                    neb = io.tile([128, 512], f16, tag="neb")
                    nc.sync.dma_start(
                        neb[:], NEB[mt * 128:(mt + 1) * 128, c * 512:(c + 1) * 512])
                    eft = io.tile([128, 512], f16, tag="eft")
                    nc.sync.dma_start(
                        eft[:], EF[mt * 128:(mt + 1) * 128, c * 512:(c + 1) * 512])
                    efc = io.tile([128, 512], f32, tag="efc")
                    nc.scalar.copy(efc[:], eft[:])
                    t1 = io.tile([128, 512], f32, tag="t1")
                    nc.vector.scalar_tensor_tensor(
                        out=t1[:], in0=wst[:, mt, c, :], scalar=rnb4[:, mt:mt + 1],
                        in1=neb[:], op0=OP.mult, op1=OP.add)
                    nc.vector.tensor_scalar(
                        out=t1[:], in0=t1[:], scalar1=1.0 - EPS_CLAMP,
                        scalar2=EPS_CLAMP, op0=OP.min, op1=OP.max)
                    pt = io.tile([128, 512], f32, tag="pt")
                    nc.vector.tensor_tensor(
                        out=pt[:], in0=t1[:], in1=efc[:], op=OP.mult)
                    nd = io.tile([128, 512], f32, tag="efc")
                    nc.vector.scalar_tensor_tensor(
                        out=nd[:], in0=t1[:], scalar=1.0, in1=pt[:],
                        op0=OP.subtract, op1=OP.subtract)
                    rc = io.tile([128, 512], f32, tag="t1")
                    nc.vector.reciprocal(rc[:], nd[:])
                    q1 = io.tile([128, 512], f32, tag="pt")
                    nc.vector.tensor_tensor(
                        out=q1[:], in0=pt[:], in1=rc[:], op=OP.mult)
                    msk = io.tile([128, 512], f32, tag="msk")
                    nc.vector.tensor_scalar(
                        out=msk[:], in0=q1[:], scalar1=sm[:, 12:13],
                        scalar2=None, op0=OP.is_lt)
                    nc.vector.scalar_tensor_tensor(
                        out=wst[:, mt, c, :], in0=q1[:], scalar=-1.0,
                        in1=msk[:], op0=OP.mult, op1=OP.mult)

            # ---------------- column-degree partials -> RS ------------------
            for c in range(NCC):
                psd = ps.tile([1, 512], f32, tag="ps", name=f"psd{c}")
                for mt in range(NMT):
                    nc.tensor.matmul(psd[:], onescol[:], wst[:, mt, c, :],
                                     start=(mt == 0), stop=(mt == NMT - 1))
                degc = io.tile([1, 512], f32, tag="degc", bufs=2)
                nc.scalar.copy(degc[:], psd[:])
                nc.sync.dma_start(degb[c * 512:(c + 1) * 512], degc[:])
            nc.gpsimd.collective_compute(
                "ReduceScatter", OP.add, replica_groups=rg,
                ins=[degb[:].opt()], outs=[degs[:].opt()],
            )
            degl = sb.tile([128, 4], f32)
            nc.sync.dma_start(degl[:], degs.rearrange("(t p) -> p t", p=128))
            degf = sb.tile([128, 4], f32)
            nc.vector.tensor_tensor(out=degf[:], in0=degl[:], in1=dl4[:], op=OP.add)
            dsq = sb.tile([128, 4], f32)
            nc.scalar.sqrt(dsq[:], degf[:])
            dis4 = sb.tile([128, 4], f32)
            nc.vector.reciprocal(dis4[:], dsq[:])

            # ---------------- Y1 scaled (in place, y1s filled in phase 0) ---
            for mt in range(NMT):
                nc.scalar.activation(y1s[:, mt, :], y1s[:, mt, :], AF.Copy,
                                     scale=dis4[:, mt:mt + 1])

            # ---------------- layer-1 aggregation partials -> RS ------------
            for c in range(NCC):
                for w in range(4):
                    psa = ps.tile([128, 1024], f32, tag="ps", name=f"psa{c}_{w}")
                    for mt in range(NMT):
                        lhsT = wst[:, mt, c, w * 128:(w + 1) * 128]
                        for nn in range(2):
                            nc.tensor.matmul(
                                psa[:, nn * 512:(nn + 1) * 512], lhsT,
                                y1s[:, mt, nn * 512:(nn + 1) * 512],
                                start=(mt == 0), stop=(mt == NMT - 1))
                    o1 = io.tile([128, 1024], f16, tag="o1")
                    nc.scalar.copy(o1[:], psa[:])
                    nc.sync.dma_start(
                        pb[c * 512 + w * 128:c * 512 + (w + 1) * 128, :], o1[:])
            nc.gpsimd.collective_compute(
                "ReduceScatter", OP.add, replica_groups=rg,
                ins=[pb[:].opt()], outs=[rs1[:].opt()],
            )

            # ---------------- layer-1 post: h = relu(dis*(S + delta*Y1s) + b0)
            h16 = sb.tile([128, NMT, 1024], f16)
            for mt in range(NMT):
                rst = io.tile([128, 1024], f16, tag="rst", bufs=2)
                nc.sync.dma_start(rst[:], rs1[mt * 128:(mt + 1) * 128, :])
                s1 = io.tile([128, 1024], f32, tag="s1", bufs=2)
                nc.vector.scalar_tensor_tensor(
                    out=s1[:], in0=y1s[:, mt, :], scalar=dl4[:, mt:mt + 1],
                    in1=rst[:], op0=OP.mult, op1=OP.add)
                nc.vector.scalar_tensor_tensor(
                    out=s1[:], in0=s1[:], scalar=dis4[:, mt:mt + 1],
                    in1=b0bc[:], op0=OP.mult, op1=OP.add)
                nc.scalar.activation(h16[:, mt, :], s1[:], AF.Relu)

            # ---------------- Y2 = (h @ blockdiag(mW1,sW1)) * dis -----------
            wbt = sb.tile([128, 8, 512], f16)
            for kb in range(8):
                nc.sync.dma_start(wbt[:, kb, :], wbout[kb, :, :])
            y2s = sb.tile([128, NMT, 512], f16)
            for mt in range(NMT):
                htb = io.tile([128, 8, 128], f16, tag="htb", bufs=2)
                for kb in range(8):
                    pst = ps.tile([128, 128], f16, tag="ps", name=f"pst{mt}_{kb}")
                    nc.tensor.transpose(
                        pst[:], h16[:, mt, kb * 128:(kb + 1) * 128], ident[:])
                    nc.scalar.copy(htb[:, kb, :], pst[:])
                psy = ps.tile([128, 512], f32, tag="ps", name=f"psy{mt}")
                for kb in range(8):
                    nc.tensor.matmul(psy[:], htb[:, kb, :], wbt[:, kb, :],
                                     start=(kb == 0), stop=(kb == 7))
                nc.scalar.activation(y2s[:, mt, :], psy[:], AF.Copy,
                                     scale=dis4[:, mt:mt + 1])

            # ---------------- layer-2 aggregation partials -> RS ------------
            for c in range(NCC):
                for w in range(4):
                    ps2 = ps.tile([128, 512], f32, tag="ps", name=f"ps2{c}_{w}")
                    for mt in range(NMT):
                        nc.tensor.matmul(
                            ps2[:], wst[:, mt, c, w * 128:(w + 1) * 128],
                            y2s[:, mt, :], start=(mt == 0), stop=(mt == NMT - 1))
                    o2 = io.tile([128, 512], f16, tag="o2")
                    nc.scalar.copy(o2[:], ps2[:])
                    nc.sync.dma_start(
                        p2b[c * 512 + w * 128:c * 512 + (w + 1) * 128, :], o2[:])
            nc.gpsimd.collective_compute(
                "ReduceScatter", OP.add, replica_groups=rg,
                ins=[p2b[:].opt()], outs=[rs2[:].opt()],
            )

            # ---------------- layer-2 post + output -------------------------
            for mt in range(NMT):
                r2t = io.tile([128, 512], f16, tag="r2t", bufs=2)
                nc.sync.dma_start(r2t[:], rs2[mt * 128:(mt + 1) * 128, :])
                z1 = io.tile([128, 512], f32, tag="z1", bufs=2)
                nc.vector.scalar_tensor_tensor(
                    out=z1[:], in0=y2s[:, mt, :], scalar=dl4[:, mt:mt + 1],
                    in1=r2t[:], op0=OP.mult, op1=OP.add)
                nc.vector.scalar_tensor_tensor(
                    out=z1[:], in0=z1[:], scalar=dis4[:, mt:mt + 1],
                    in1=b1bc[:], op0=OP.mult, op1=OP.add)
                zo = io.tile([128, 512], f16, tag="zo", bufs=2)
                nc.scalar.activation(zo[:], z1[:], AF.Relu)
                nc.sync.dma_start(ZOUT[mt * 128:(mt + 1) * 128, :], zo[:])
    return nc


def _run(name, builder, in_maps):
    global _LAST_DEVICE_WALL
    if name not in _CACHE:
        _CACHE[name] = builder()
    t0 = time.time()
    res = run_bass_kernel_spmd(_CACHE[name], in_maps, core_ids=list(range(NC)))
    dt = time.time() - t0
    _LAST_DEVICE_WALL += dt
    import os
    if os.environ.get("KERNEL_DEBUG_TIMES"):
        print(f"  [dev call {name}: {dt * 1e3:.1f} ms]")
    return res.results


# ---------------------------------------------------------------------------
def _l2n(x):
    n = np.sqrt(np.sum(x * x, axis=1, keepdims=True))
    return x / np.maximum(n, 1e-12)


def kernel(x, new_edge, beta, delta, eps, Wm, bm, Ws, bs,
           mW0, mb0, mW1, mb1, sW0, sb0, sW1, sb1):
    global _LAST_DEVICE_WALL
    _LAST_DEVICE_WALL = 0.0
    x = np.asarray(x, np.float32)
    b = float(np.asarray(beta).reshape(-1)[0])
    d = float(np.asarray(delta).reshape(-1)[0])

    x_mean = x @ Wm + bm
    x_std = x @ Ws + bs

    m = _l2n(x_mean)
    c = _l2n(np.exp(x_std))
    cs = np.sqrt(c)
    sq = np.sum(m * m, axis=1)
    csum = np.sum(c, axis=1)
    u = (sq + csum).astype(np.float32)

    g = np.concatenate([m, cs], axis=1).astype(np.float32)  # [N, 1024]

    ne = np.asarray(new_edge, np.float32)
    NEBf = (b * ne).astype(np.float16)
    e = np.clip(np.asarray(eps, np.float32), EPS_CLAMP, 1.0 - EPS_CLAMP)
    Ef = np.minimum(e / (1.0 - e), 60000.0).astype(np.float16)

    WBD = np.zeros((1024, 512), np.float16)
    WBD[:512, :256] = mW1
    WBD[512:, 256:] = sW1

    BRv = np.concatenate([mb0, sb0, mb1, sb1]).astype(np.float16).reshape(1, 1536)
    # packed projection weights + folded layer-0 weights / bias rows
    WCAT = np.concatenate(
        [Wm, Ws, Wm @ mW0, Ws @ sW0], axis=1).astype(np.float16)  # [512, 2048]
    BR2v = np.concatenate(
        [bm, bs, bm @ mW0, bs @ sW0]).astype(np.float32).reshape(1, 2048)
    x16 = x.astype(np.float16)
    XI_cat = np.ascontiguousarray(
        x16.reshape(NC, BLK, F).transpose(0, 2, 1).reshape(NC * F, BLK))

    # host diagonal scalars (match the on-device dataflow, f32 approx)
    g2 = np.sum(g * g, axis=1)
    wsdiag = np.exp(-(2.0 * u - 2.0 * g2)).astype(np.float32)
    nebdiag = np.diagonal(NEBf).astype(np.float32)
    Ediag = np.diagonal(Ef).astype(np.float32)

    # pre-concatenated (all-cores) inputs: axis0 = core-major, zero extra
    # copies for the row-sharded tensors
    SM_cat = np.zeros((NC, 128, 16), np.float32)
    SM_cat[:, :, 0:4] = wsdiag.reshape(NC, 4, 128).transpose(0, 2, 1)
    SM_cat[:, :, 4:8] = nebdiag.reshape(NC, 4, 128).transpose(0, 2, 1)
    SM_cat[:, :, 8:12] = Ediag.reshape(NC, 4, 128).transpose(0, 2, 1)
    SM_cat[:, :, 12] = -d
    SM_cat[:, :, 13] = 1.0 - b
    SM_cat = SM_cat.reshape(NC * 128, 16)
    concat_map = {
        "XI": XI_cat,
        "WCK": WCAT,
        "BR2": np.repeat(BR2v, NC, axis=0),
        "NEB": NEBf,
        "EF": Ef,
        "WBK": WBD,
        "BR": np.repeat(BRv, NC, axis=0),
        "SM": SM_cat,
    }

    res = _run("fused", _build_fused, [{"__concat__": concat_map}])
    z = np.concatenate([res[k]["ZOUT"] for k in range(NC)], axis=0)
    z_mean = np.ascontiguousarray(z[:, :H]).astype(np.float32)
    z_std = np.ascontiguousarray(z[:, H:2 * H]).astype(np.float32)
    return z_mean, z_std


# revision 38
# speedup vs baseline: 10.2943x; 1.2220x over previous
"""Trainium2 kernel for nn_GaussianModel (gnn_message_passing).

Single fused device call, row-sharded across 8 cores (core k owns rows
r_k = [k*512, (k+1)*512) of the 4096-node graph):

  host:   x_mean/x_std projections, l2-norms, Y1 = [xm@mW0 | xs@sW0],
          E = e/(1-e), b*new_edge  (cheap O(N*F) / elementwise prep)
  device: Gram via augmented-K matmul  q_ij = g_i.g_j - (u_i+u_j)/2
          ws = exp(2q), row-l2-norm, logit-mix with new_edge/eps
          (algebraic sigmoid form, no log/exp), threshold -> A row-block
          column degree partials -> ReduceScatter -> dis = deg^-1/2
          layer-1 aggregation partials A_k^T @ (dis*Y1) -> ReduceScatter
          relu(+b0), on-device h @ blockdiag(mW1,sW1) via PE transpose
          layer-2 aggregation partials -> ReduceScatter -> relu(+b1)
  The GCN self-loop (add 1 where diag==0) is handled analytically: a
  per-row delta in {0,1} computed from host-supplied diagonal scalars,
  added to deg and applied as a rank-1 correction after each RS.

All per-core variation lives in the input data (no partition-id control
flow); collectives (2x AllGather, 3x ReduceScatter) handle placement.
"""
import json
import sys
import time

sys.path.insert(0, "/opt/trn_rl_repo")
import numpy as np
import concourse.bass as bass
import concourse.mybir as mybir
from concourse.tile import TileContext
from concourse.bass_utils import run_bass_kernel_spmd

NC = 8
N, F, H = 4096, 512, 256
BLK = N // NC  # 512 rows per core
EPS_CLAMP = 1e-6
f32, f16 = mybir.dt.float32, mybir.dt.float16
AF = mybir.ActivationFunctionType
OP = mybir.AluOpType

# ---------------------------------------------------------------------------
# walrus in this container caps sem-waits at 1 per instruction; Tile emits
# more. Split excess waits onto preceding same-engine Drains in the BIR JSON.
_MAX_WAITS = 1


def _fix_bir_bytes(bir_json):
    j = json.loads(bir_json)
    changed = False
    for fn in j.get("functions", []):
        for bb in fn.get("blocks", []):
            new_insts = []
            for inst in bb.get("instructions", []):
                si = inst.get("sync_info") or {}
                waits = si.get("on_wait") or []
                if len(waits) > _MAX_WAITS and inst.get("engine", "Unassigned") != "Unassigned":
                    changed = True
                    keep = waits[-_MAX_WAITS:]
                    extra = waits[:-_MAX_WAITS]
                    for gi in range(0, len(extra), _MAX_WAITS):
                        new_insts.append({
                            "debug": inst.get("debug", 0),
                            "engine": inst["engine"],
                            "ins": [],
                            "outs": [],
                            "name": f"{inst['name']}-ws{gi}",
                            "opcode": "Drain",
                            "sync_info": {"on_update": [],
                                          "on_wait": extra[gi:gi + _MAX_WAITS]},
                        })
                    si = dict(si)
                    si["on_wait"] = keep
                    inst = dict(inst)
                    inst["sync_info"] = si
                new_insts.append(inst)
            bb["instructions"] = new_insts
    return json.dumps(j).encode() if changed else bir_json


def _install_birfix():
    import concourse.bass_utils as bu
    if getattr(bu, "_birfix_installed", False):
        return
    orig = bu.compile_bir_kernel

    def patched(bir_json, tmpdir, neff_name="file.neff"):
        try:
            bir_json = _fix_bir_bytes(bir_json)
        except Exception as e:
            print("birfix failed:", e)
        return orig(bir_json, tmpdir, neff_name=neff_name)

    bu.compile_bir_kernel = patched
    try:
        import concourse.bass2jax as b2j
        b2j.compile_bir_kernel = patched
    except Exception as e:
        print("birfix bass2jax hook failed:", e)
    bu._birfix_installed = True


_install_birfix()


# ---------------------------------------------------------------------------
# run_bass_via_pjrt rebuilds jax.jit + recompiles the NEFF on EVERY call.
# Memoize the jitted sharded executable per Bass object so warm calls only
# pay input transfer + device execution.
def _install_pjrt_cache():
    import concourse.bass2jax as b2j

    if getattr(b2j, "_pjrt_cache_installed", False):
        return
    import jax
    from jax.sharding import Mesh, PartitionSpec
    from jax.experimental.shard_map import shard_map

    _runners = {}

    def _build_runner(nc, n_cores):
        b2j.install_neuronx_cc_hook()
        partition_name = (
            nc.partition_id_tensor.name if nc.partition_id_tensor else None
        )
        in_names, out_names, out_avals = [], [], []
        for alloc in nc.m.functions[0].allocations:
            if not isinstance(alloc, mybir.MemoryLocationSet):
                continue
            name = alloc.memorylocations[0].name
            if alloc.kind == "ExternalInput":
                if name != partition_name:
                    in_names.append(name)
            elif alloc.kind == "ExternalOutput":
                out_names.append(name)
                out_avals.append(
                    jax.core.ShapedArray(
                        tuple(alloc.tensor_shape), mybir.dt.np(alloc.dtype)
                    )
                )
        n_params = len(in_names)
        n_outs = len(out_avals)
        all_in = in_names + out_names + ([partition_name] if partition_name else [])
        donate = tuple(range(n_params, n_params + n_outs))

        def _body(*args):
            operands = list(args)
            if partition_name is not None:
                operands.append(b2j.partition_id_tensor())
            outs = b2j._bass_exec_p.bind(
                *operands,
                out_avals=tuple(out_avals),
                in_names=tuple(all_in),
                out_names=tuple(out_names),
                lowering_input_output_aliases=(),
                sim_require_finite=True,
                sim_require_nnan=True,
                nc=nc,
            )
            return tuple(outs)

        devices = jax.devices()[:n_cores]
        mesh = Mesh(np.asarray(devices), ("core",))
        spec = PartitionSpec("core")
        sharded = jax.jit(
            shard_map(
                _body,
                mesh=mesh,
                in_specs=(spec,) * (n_params + n_outs),
                out_specs=(spec,) * n_outs,
                check_rep=False,
            ),
            donate_argnums=donate,
            keep_unused=True,
        )
        # donated output buffers created on-device (no host->device zeros)
        import jax.numpy as jnp
        from jax.sharding import NamedSharding

        zshard = NamedSharding(mesh, spec)
        zfns = [
            jax.jit(
                (lambda shape, dtype: (lambda: jnp.zeros(shape, dtype)))(
                    (n_cores * av.shape[0], *av.shape[1:]), av.dtype
                ),
                out_shardings=zshard,
            )
            for av in out_avals
        ]

        b2j._pjrt_internals = {
            "sharded": sharded, "in_names": in_names, "out_names": out_names,
            "out_avals": out_avals, "mesh": mesh, "spec": spec, "zfns": zfns,
        }

        def run(in_maps=None, concat=None):
            if concat is not None:
                concat_in = [
                    np.ascontiguousarray(concat[name]) for name in in_names
                ]
            else:
                if nc.dbg_addr is not None:
                    in_maps = [
                        {**m, nc.dbg_addr.name: np.zeros((1, 2), np.uint32)}
                        for m in in_maps
                    ]
                per_core = [
                    [np.asarray(m[name]) for name in in_names] for m in in_maps
                ]
                concat_in = [
                    np.concatenate([per_core[c][i] for c in range(n_cores)], axis=0)
                    for i in range(n_params)
                ]
            concat_zeros = [zf() for zf in zfns]
            out_arrs = sharded(*concat_in, *concat_zeros)
            return [
                {
                    name: np.asarray(out_arrs[i]).reshape(
                        n_cores, *out_avals[i].shape
                    )[c]
                    for i, name in enumerate(out_names)
                }
                for c in range(n_cores)
            ]

        return run

    def cached(nc, in_maps, n_cores):
        key = (id(nc), n_cores)
        if key not in _runners:
            _runners[key] = _build_runner(nc, n_cores)
        if len(in_maps) == 1 and "__concat__" in in_maps[0]:
            return _runners[key](concat=in_maps[0]["__concat__"])
        return _runners[key](in_maps)

    b2j.run_bass_via_pjrt = cached
    b2j._pjrt_cache_installed = True


_install_pjrt_cache()

# ---------------------------------------------------------------------------
_CACHE = {}
_LAST_DEVICE_WALL = 0.0

NMT = 4   # row-tiles of 128 per core
NCC = 8   # column chunks of 512
NKC = 9   # K-tiles of 128 (1024 g-features + 4 augmented rows, padded)
KAUG = 1152  # 9 * 128


def _build_fused():
    nc = bass.Bass("TRN2", num_devices=NC)
    # x.T column-slice for this core's rows: [infeat 512, nodes 512]
    XI = nc.dram_tensor("XI", [F, BLK], f16, kind="ExternalInput")
    # [Wm | Ws | Wm@mW0 | Ws@sW0] row-shard: [64, 2048] (AllGathered)
    WCK = nc.dram_tensor("WCK", [F // NC, 4 * F], f16, kind="ExternalInput")
    # bias rows: [bm | bs | bm@mW0 | bs@sW0], [1, 2048]
    BR2 = nc.dram_tensor("BR2", [1, 4 * F], f32, kind="ExternalInput")
    NEBQ = nc.dram_tensor("NEBQ", [BLK, N], mybir.dt.uint8, kind="ExternalInput")
    EQ = nc.dram_tensor("EQ", [BLK, N], mybir.dt.uint16, kind="ExternalInput")
    WBK = nc.dram_tensor("WBK", [128, 512], f16, kind="ExternalInput")
    BR = nc.dram_tensor("BR", [1, 1536], f16, kind="ExternalInput")
    SM = nc.dram_tensor("SM", [128, 16], f32, kind="ExternalInput")
    ZOUT = nc.dram_tensor("ZOUT", [BLK, 512], f16, kind="ExternalOutput")

    identity_np = np.eye(128, dtype=np.float16)
    ones_col_np = np.ones((128, 1), dtype=np.float16)
    ones_row_np = np.ones((1, 128), dtype=np.float16)
    augones_np = np.zeros((4, BLK), dtype=np.float16)
    augones_np[2:4] = 1.0
    IDC = nc.inline_tensor(identity_np, name="idc")
    OCC = nc.inline_tensor(ones_col_np, name="occ")
    ORC = nc.inline_tensor(ones_row_np, name="orc")
    AUGO = nc.inline_tensor(augones_np, name="augo")

    rg = [list(range(NC))]

    with TileContext(nc) as tc:
        with (
            tc.tile_pool(name="sb", bufs=1) as sb,
            tc.tile_pool(name="rfp", bufs=2) as rfp,
            tc.tile_pool(name="io", bufs=3) as io,
            tc.tile_pool(name="ps", bufs=4, space="PSUM") as ps,
            tc.tile_pool(name="dr", bufs=1, space="DRAM") as dr,
        ):
            # ---------------- DRAM scratch + collectives wiring ------------
            rgin = dr.tile([KAUG, BLK], f16)
            rgout = dr.tile([NC, KAUG, BLK], f16, addr_space="Shared")
            wcin = dr.tile([F // NC, 4 * F], f16)
            wcout = dr.tile([NC, F // NC, 4 * F], f16, addr_space="Shared")
            wbin = dr.tile([128, 512], f16)
            wbout = dr.tile([NC, 128, 512], f16, addr_space="Shared")
            degb = dr.tile([N], f32)
            degs = dr.tile([BLK], f32)
            pb = dr.tile([N, 1024], f16)
            rs1 = dr.tile([BLK, 1024], f16)
            p2b = dr.tile([N, 512], f16)
            rs2 = dr.tile([BLK, 512], f16)

            nc.sync.dma_start(wcin[:], WCK[:])
            nc.gpsimd.collective_compute(
                "AllGather", OP.bypass, replica_groups=rg,
                ins=[wcin[:].opt()], outs=[wcout[:].opt()],
            )
            nc.sync.dma_start(wbin[:], WBK[:])
            nc.gpsimd.collective_compute(
                "AllGather", OP.bypass, replica_groups=rg,
                ins=[wbin[:].opt()], outs=[wbout[:].opt()],
            )

            # ---------------- constants / scalars --------------------------
            ident = sb.tile([128, 128], f16)
            nc.sync.dma_start(ident[:], IDC[:])
            onescol = sb.tile([128, 1], f16)
            nc.sync.dma_start(onescol[:], OCC[:])
            onesrow = sb.tile([1, 128], f16)
            nc.sync.dma_start(onesrow[:], ORC[:])
            sm = sb.tile([128, 16], f32)
            nc.sync.dma_start(sm[:], SM[:])
            brs = sb.tile([1, 1536], f16)
            nc.sync.dma_start(brs[:], BR[:])
            # BR2 arrives f32; load then narrow to f16 for the K=1 matmuls
            br2f = sb.tile([1, 4 * F], f32)
            nc.sync.dma_start(br2f[:], BR2[:])
            br2h = sb.tile([1, 4 * F], f16)
            nc.scalar.copy(br2h[:], br2f[:])

            # bias broadcast via K=1 matmul: ones_col x bias_row
            b0bc = sb.tile([128, 1024], f32)
            b1bc = sb.tile([128, 512], f32)
            for j in range(3):
                psb = ps.tile([128, 512], f32, tag="ps", name=f"psb{j}")
                nc.tensor.matmul(psb[:], onesrow[:], brs[:, j * 512:(j + 1) * 512],
                                 start=True, stop=True)
                dst = b0bc[:, j * 512:(j + 1) * 512] if j < 2 else b1bc[:]
                nc.scalar.copy(dst, psb[:])

            # ---------------- x-projections, norms, g assembly -------------
            # lk[:, kc<8, :] = g.T feature-tiles (m kc 0..3, cs kc 4..7),
            # built on device from XI and the AllGathered weight blocks.
            xi = sb.tile([128, 4, BLK], f16)
            for kt in range(4):
                nc.sync.dma_start(xi[:, kt, :], XI[kt * 128:(kt + 1) * 128, :])
            wcflat = wcout.rearrange("a b c -> (a b) c")
            lk = sb.tile([128, NKC, BLK], f16)
            u4 = sb.tile([128, 4], f32)
            y1s = sb.tile([128, NMT, 1024], f16)
            for wblk in range(4):
                wcb = io.tile([128, 4, 512], f16, tag="wcb", bufs=2)
                for kt in range(4):
                    nc.sync.dma_start(
                        wcb[:, kt, :],
                        wcflat[kt * 128:(kt + 1) * 128,
                               wblk * 512:(wblk + 1) * 512])
                for mt in range(NMT):
                    pm = ps.tile([128, 512], f32, tag="ps", name=f"pm{wblk}_{mt}")
                    for kt in range(4):
                        nc.tensor.matmul(
                            pm[:], xi[:, kt, mt * 128:(mt + 1) * 128],
                            wcb[:, kt, :], start=(kt == 0), stop=False)
                    nc.tensor.matmul(
                        pm[:], onesrow[:],
                        br2h[:, wblk * 512:(wblk + 1) * 512],
                        start=False, stop=True)
                    if wblk == 0:
                        # m = l2-normalized x_mean, transposed into lk
                        xmn = io.tile([128, 512], f32, tag="xmn", bufs=2)
                        nc.scalar.copy(xmn[:], pm[:])
                        junk2 = io.tile([128, 512], f16, tag="junk", bufs=2)
                        sqx = io.tile([128, 1], f32, tag="sqx")
                        nc.scalar.activation(junk2[:], xmn[:], AF.Square,
                                             accum_out=sqx[:])
                        srx = io.tile([128, 1], f32, tag="srx")
                        nc.scalar.sqrt(srx[:], sqx[:])
                        rnx = io.tile([128, 1], f32, tag="rnx")
                        nc.vector.reciprocal(rnx[:], srx[:])
                        mtile = io.tile([128, 512], f16, tag="mtile", bufs=2)
                        nc.scalar.activation(mtile[:], xmn[:], AF.Copy,
                                             scale=rnx[:])
                        for ftb in range(4):
                            ptr = ps.tile([128, 128], f16, tag="ps",
                                          name=f"ptm{mt}_{ftb}")
                            nc.tensor.transpose(
                                ptr[:], mtile[:, ftb * 128:(ftb + 1) * 128],
                                ident[:])
                            nc.scalar.copy(
                                lk[:, ftb, mt * 128:(mt + 1) * 128], ptr[:])
                    elif wblk == 1:
                        # c = l2n(exp(x_std)); cs = sqrt(c); u = 1 + sum(c)
                        c0 = io.tile([128, 512], f32, tag="xmn", bufs=2)
                        cs0 = io.tile([128, 1], f32, tag="cs0")
                        nc.scalar.activation(c0[:], pm[:], AF.Exp,
                                             accum_out=cs0[:])
                        junk2 = io.tile([128, 512], f16, tag="junk", bufs=2)
                        sqx = io.tile([128, 1], f32, tag="sqx")
                        nc.scalar.activation(junk2[:], c0[:], AF.Square,
                                             accum_out=sqx[:])
                        srx = io.tile([128, 1], f32, tag="srx")
                        nc.scalar.sqrt(srx[:], sqx[:])
                        rnx = io.tile([128, 1], f32, tag="rnx")
                        nc.vector.reciprocal(rnx[:], srx[:])
                        cstile = io.tile([128, 512], f16, tag="mtile", bufs=2)
                        nc.scalar.activation(cstile[:], c0[:], AF.Sqrt,
                                             scale=rnx[:])
                        for ftb in range(4):
                            ptr = ps.tile([128, 128], f16, tag="ps",
                                          name=f"ptc{mt}_{ftb}")
                            nc.tensor.transpose(
                                ptr[:], cstile[:, ftb * 128:(ftb + 1) * 128],
                                ident[:])
                            nc.scalar.copy(
                                lk[:, 4 + ftb, mt * 128:(mt + 1) * 128],
                                ptr[:])
                        nc.vector.tensor_tensor(
                            out=u4[:, mt:mt + 1], in0=cs0[:], in1=rnx[:],
                            op=OP.mult)
                    else:
                        # Y1 halves (x @ (W @ W0) + fold(b)), unscaled yet
                        half = wblk - 2
                        nc.scalar.copy(
                            y1s[:, mt, half * 512:(half + 1) * 512], pm[:])

            # u = 1 + csum; v = -u/2 split into f16 hi/lo
            nc.vector.tensor_scalar(out=u4[:], in0=u4[:], scalar1=1.0,
                                    scalar2=None, op0=OP.add)
            vneg = sb.tile([128, 4], f32)
            nc.vector.tensor_scalar(out=vneg[:], in0=u4[:], scalar1=-0.5,
                                    scalar2=None, op0=OP.mult)
            vh4 = sb.tile([128, 4], f16)
            nc.scalar.copy(vh4[:], vneg[:])
            vh4f = sb.tile([128, 4], f32)
            nc.scalar.copy(vh4f[:], vh4[:])
            vl4 = sb.tile([128, 4], f16)
            nc.vector.tensor_tensor(out=vl4[:], in0=vneg[:], in1=vh4f[:],
                                    op=OP.subtract)

            # lk K-tile 8 (lhsT aug rows [vhi; vlo; 1; 1]) + rgin assembly
            nc.vector.memset(lk[:, 8, :], 0.0)
            nc.sync.dma_start(lk[0:4, 8, :], AUGO[:])
            pag = ps.tile([2, 512], f16, tag="ps", name="pag")
            for t in range(4):
                vhl = io.tile([128, 2], f16, tag="vhl", bufs=2)
                nc.vector.tensor_copy(vhl[:, 0:1], vh4[:, t:t + 1])
                nc.vector.tensor_copy(vhl[:, 1:2], vl4[:, t:t + 1])
                nc.tensor.transpose(pag[0:2, t * 128:(t + 1) * 128],
                                    vhl[:], ident[:])
            nc.scalar.copy(lk[0:2, 8, :], pag[:])

            # rgin (rhs side, aug rows swapped: [1; 1; vhi; vlo]) -> AllGather
            for kc in range(8):
                nc.sync.dma_start(rgin[kc * 128:(kc + 1) * 128, :], lk[:, kc, :])
            onetile = sb.tile([2, BLK], f16)
            nc.vector.memset(onetile[:], 1.0)
            nc.sync.dma_start(rgin[1024:1026, :], onetile[:])
            nc.sync.dma_start(rgin[1026:1028, :], lk[0:2, 8, :])
            ztile = sb.tile([124, BLK], f16)
            nc.vector.memset(ztile[:], 0.0)
            nc.sync.dma_start(rgin[1028:1152, :], ztile[:])
            nc.gpsimd.collective_compute(
                "AllGather", OP.bypass, replica_groups=rg,
                ins=[rgin[:].opt()], outs=[rgout[:].opt()],
            )

            # ---------------- gram + ws = exp(2q) ---------------------------
            # wst holds ws (f16) per (mt, c) chunk; the elementwise chain
            # later overwrites each chunk in place with the final A values.
            wst = sb.tile([128, NMT, NCC, 512], f16)
            sqa = sb.tile([128, 32], f32)
            for c in range(NCC):
                rf = rfp.tile([128, NKC, 512], f16, tag="rf")
                for kc in range(NKC):
                    nc.sync.dma_start(
                        rf[:, kc, :], rgout[c, kc * 128:(kc + 1) * 128, :])
                for mt in range(NMT):
                    psg = ps.tile([128, 512], f32, tag="ps", name=f"psg{c}_{mt}")
                    for kc in range(NKC):
                        nc.tensor.matmul(
                            psg[:], lk[:, kc, mt * 128:(mt + 1) * 128],
                            rf[:, kc, :], start=(kc == 0), stop=(kc == NKC - 1))
                    wsl = wst[:, mt, c, :]
                    nc.scalar.activation(wsl, psg[:], AF.Exp, scale=2.0)
                    junk = io.tile([128, 512], f16, tag="junk", bufs=2)
                    nc.scalar.activation(
                        junk[:], wsl, AF.Square,
                        accum_out=sqa[:, mt * 8 + c:mt * 8 + c + 1])

            # ---------------- row norms + delta (self-loop indicator) ------
            rn4 = sb.tile([128, 4], f32)     # 1/||ws_row||
            rnb4 = sb.tile([128, 4], f32)    # (1-b) * rn4
            dl4 = sb.tile([128, 4], f32)     # delta in {0,1}
            for mt in range(NMT):
                rtmp = io.tile([128, 1], f32, tag="rtmp")
                nc.vector.tensor_reduce(
                    rtmp[:], sqa[:, mt * 8:(mt + 1) * 8],
                    axis=mybir.AxisListType.X, op=OP.add)
                rsq = io.tile([128, 1], f32, tag="rsq")
                nc.scalar.sqrt(rsq[:], rtmp[:])
                nc.vector.reciprocal(rn4[:, mt:mt + 1], rsq[:])
                nc.vector.tensor_scalar(
                    out=rnb4[:, mt:mt + 1], in0=rn4[:, mt:mt + 1],
                    scalar1=sm[:, 13:14], scalar2=None, op0=OP.mult)
                # delta chain on [128,1] diagonal scalars
                wsn = io.tile([128, 1], f32, tag="wsn")
                nc.vector.tensor_tensor(
                    out=wsn[:], in0=sm[:, mt:mt + 1], in1=rn4[:, mt:mt + 1],
                    op=OP.mult)
                t1d = io.tile([128, 1], f32, tag="t1d")
                nc.vector.scalar_tensor_tensor(
                    out=t1d[:], in0=wsn[:], scalar=sm[:, 13:14],
                    in1=sm[:, 4 + mt:5 + mt], op0=OP.mult, op1=OP.add)
                nc.vector.tensor_scalar(
                    out=t1d[:], in0=t1d[:], scalar1=1.0 - EPS_CLAMP,
                    scalar2=EPS_CLAMP, op0=OP.min, op1=OP.max)
                pd = io.tile([128, 1], f32, tag="pd")
                nc.vector.tensor_tensor(
                    out=pd[:], in0=t1d[:], in1=sm[:, 8 + mt:9 + mt], op=OP.mult)
                ndd = io.tile([128, 1], f32, tag="ndd")
                nc.vector.scalar_tensor_tensor(
                    out=ndd[:], in0=t1d[:], scalar=1.0, in1=pd[:],
                    op0=OP.subtract, op1=OP.subtract)
                rdd = io.tile([128, 1], f32, tag="rdd")
                nc.vector.reciprocal(rdd[:], ndd[:])
                qdd = io.tile([128, 1], f32, tag="qdd")
                nc.vector.tensor_tensor(
                    out=qdd[:], in0=pd[:], in1=rdd[:], op=OP.mult)
                nc.vector.tensor_scalar(
                    out=dl4[:, mt:mt + 1], in0=qdd[:], scalar1=sm[:, 12:13],
                    scalar2=None, op0=OP.is_ge)

            # ---------------- elementwise chain -> A (f16, in wst) ----------
            # term = clip((1-b)*ws_n + b*ne); A = sig(logit(term)+logit(e))
            # with sig(logit(t)+logit(e)) = t*E/(t*E + 1 - t),  E = e/(1-e)
            for c in range(NCC):
                for mt in range(NMT):
                    neb = io.tile([128, 512], mybir.dt.uint8, tag="neb")
                    nc.sync.dma_start(
                        neb[:], NEBQ[mt * 128:(mt + 1) * 128, c * 512:(c + 1) * 512])
                    eft = io.tile([128, 512], mybir.dt.uint16, tag="eft")
                    nc.sync.dma_start(
                        eft[:], EQ[mt * 128:(mt + 1) * 128, c * 512:(c + 1) * 512])
                    # dequant: neb_f = q * (b/255)  [per-partition scalar AP]
                    nebf = io.tile([128, 512], f16, tag="nebf")
                    nc.vector.tensor_scalar(
                        out=nebf[:], in0=neb[:], scalar1=sm[:, 14:15],
                        scalar2=None, op0=OP.mult)
                    # e = clip(q/65535); E = e/(1-e) in f32
                    ef_ = io.tile([128, 512], f32, tag="ef32")
                    nc.vector.tensor_scalar(
                        out=ef_[:], in0=eft[:], scalar1=1.0 / 65535.0,
                        scalar2=None, op0=OP.mult)
                    nc.vector.tensor_scalar(
                        out=ef_[:], in0=ef_[:], scalar1=1.0 - EPS_CLAMP,
                        scalar2=EPS_CLAMP, op0=OP.min, op1=OP.max)
                    onem = io.tile([128, 512], f32, tag="ef32")
                    nc.vector.tensor_scalar(
                        out=onem[:], in0=ef_[:], scalar1=-1.0, scalar2=1.0,
                        op0=OP.mult, op1=OP.add)
                    nc.vector.reciprocal(onem[:], onem[:])
                    efc = io.tile([128, 512], f32, tag="pt")
                    nc.vector.tensor_tensor(
                        out=efc[:], in0=ef_[:], in1=onem[:], op=OP.mult)
                    t1 = io.tile([128, 512], f32, tag="t1")
                    nc.vector.scalar_tensor_tensor(
                        out=t1[:], in0=wst[:, mt, c, :], scalar=rnb4[:, mt:mt + 1],
                        in1=nebf[:], op0=OP.mult, op1=OP.add)
                    nc.vector.tensor_scalar(
                        out=t1[:], in0=t1[:], scalar1=1.0 - EPS_CLAMP,
                        scalar2=EPS_CLAMP, op0=OP.min, op1=OP.max)
                    pt = io.tile([128, 512], f32, tag="pt")
                    nc.vector.tensor_tensor(
                        out=pt[:], in0=t1[:], in1=efc[:], op=OP.mult)
                    nd = io.tile([128, 512], f32, tag="efc")
                    nc.vector.scalar_tensor_tensor(
                        out=nd[:], in0=t1[:], scalar=1.0, in1=pt[:],
                        op0=OP.subtract, op1=OP.subtract)
                    rc = io.tile([128, 512], f32, tag="t1")
                    nc.vector.reciprocal(rc[:], nd[:])
                    q1 = io.tile([128, 512], f32, tag="pt")
                    nc.vector.tensor_tensor(
                        out=q1[:], in0=pt[:], in1=rc[:], op=OP.mult)
                    msk = io.tile([128, 512], f32, tag="msk")
                    nc.vector.tensor_scalar(
                        out=msk[:], in0=q1[:], scalar1=sm[:, 12:13],
                        scalar2=None, op0=OP.is_lt)
                    nc.vector.scalar_tensor_tensor(
                        out=wst[:, mt, c, :], in0=q1[:], scalar=-1.0,
                        in1=msk[:], op0=OP.mult, op1=OP.mult)

            # ---------------- column-degree partials -> RS ------------------
            for c in range(NCC):
                psd = ps.tile([1, 512], f32, tag="ps", name=f"psd{c}")
                for mt in range(NMT):
                    nc.tensor.matmul(psd[:], onescol[:], wst[:, mt, c, :],
                                     start=(mt == 0), stop=(mt == NMT - 1))
                degc = io.tile([1, 512], f32, tag="degc", bufs=2)
                nc.scalar.copy(degc[:], psd[:])
                nc.sync.dma_start(degb[c * 512:(c + 1) * 512], degc[:])
            nc.gpsimd.collective_compute(
                "ReduceScatter", OP.add, replica_groups=rg,
                ins=[degb[:].opt()], outs=[degs[:].opt()],
            )
            degl = sb.tile([128, 4], f32)
            nc.sync.dma_start(degl[:], degs.rearrange("(t p) -> p t", p=128))
            degf = sb.tile([128, 4], f32)
            nc.vector.tensor_tensor(out=degf[:], in0=degl[:], in1=dl4[:], op=OP.add)
            dsq = sb.tile([128, 4], f32)
            nc.scalar.sqrt(dsq[:], degf[:])
            dis4 = sb.tile([128, 4], f32)
            nc.vector.reciprocal(dis4[:], dsq[:])

            # ---------------- Y1 scaled (in place, y1s filled in phase 0) ---
            for mt in range(NMT):
                nc.scalar.activation(y1s[:, mt, :], y1s[:, mt, :], AF.Copy,
                                     scale=dis4[:, mt:mt + 1])

            # ---------------- layer-1 aggregation partials -> RS ------------
            for c in range(NCC):
                for w in range(4):
                    psa = ps.tile([128, 1024], f32, tag="ps", name=f"psa{c}_{w}")
                    for mt in range(NMT):
                        lhsT = wst[:, mt, c, w * 128:(w + 1) * 128]
                        for nn in range(2):
                            nc.tensor.matmul(
                                psa[:, nn * 512:(nn + 1) * 512], lhsT,
                                y1s[:, mt, nn * 512:(nn + 1) * 512],
                                start=(mt == 0), stop=(mt == NMT - 1))
                    o1 = io.tile([128, 1024], f16, tag="o1")
                    nc.scalar.copy(o1[:], psa[:])
                    nc.sync.dma_start(
                        pb[c * 512 + w * 128:c * 512 + (w + 1) * 128, :], o1[:])
            nc.gpsimd.collective_compute(
                "ReduceScatter", OP.add, replica_groups=rg,
                ins=[pb[:].opt()], outs=[rs1[:].opt()],
            )

            # ---------------- layer-1 post: h = relu(dis*(S + delta*Y1s) + b0)
            h16 = sb.tile([128, NMT, 1024], f16)
            for mt in range(NMT):
                rst = io.tile([128, 1024], f16, tag="rst", bufs=2)
                nc.sync.dma_start(rst[:], rs1[mt * 128:(mt + 1) * 128, :])
                s1 = io.tile([128, 1024], f32, tag="s1", bufs=2)
                nc.vector.scalar_tensor_tensor(
                    out=s1[:], in0=y1s[:, mt, :], scalar=dl4[:, mt:mt + 1],
                    in1=rst[:], op0=OP.mult, op1=OP.add)
                nc.vector.scalar_tensor_tensor(
                    out=s1[:], in0=s1[:], scalar=dis4[:, mt:mt + 1],
                    in1=b0bc[:], op0=OP.mult, op1=OP.add)
                nc.scalar.activation(h16[:, mt, :], s1[:], AF.Relu)

            # ---------------- Y2 = (h @ blockdiag(mW1,sW1)) * dis -----------
            wbt = sb.tile([128, 8, 512], f16)
            for kb in range(8):
                nc.sync.dma_start(wbt[:, kb, :], wbout[kb, :, :])
            y2s = sb.tile([128, NMT, 512], f16)
            for mt in range(NMT):
                htb = io.tile([128, 8, 128], f16, tag="htb", bufs=2)
                for kb in range(8):
                    pst = ps.tile([128, 128], f16, tag="ps", name=f"pst{mt}_{kb}")
                    nc.tensor.transpose(
                        pst[:], h16[:, mt, kb * 128:(kb + 1) * 128], ident[:])
                    nc.scalar.copy(htb[:, kb, :], pst[:])
                psy = ps.tile([128, 512], f32, tag="ps", name=f"psy{mt}")
                for kb in range(8):
                    nc.tensor.matmul(psy[:], htb[:, kb, :], wbt[:, kb, :],
                                     start=(kb == 0), stop=(kb == 7))
                nc.scalar.activation(y2s[:, mt, :], psy[:], AF.Copy,
                                     scale=dis4[:, mt:mt + 1])

            # ---------------- layer-2 aggregation partials -> RS ------------
            for c in range(NCC):
                for w in range(4):
                    ps2 = ps.tile([128, 512], f32, tag="ps", name=f"ps2{c}_{w}")
                    for mt in range(NMT):
                        nc.tensor.matmul(
                            ps2[:], wst[:, mt, c, w * 128:(w + 1) * 128],
                            y2s[:, mt, :], start=(mt == 0), stop=(mt == NMT - 1))
                    o2 = io.tile([128, 512], f16, tag="o2")
                    nc.scalar.copy(o2[:], ps2[:])
                    nc.sync.dma_start(
                        p2b[c * 512 + w * 128:c * 512 + (w + 1) * 128, :], o2[:])
            nc.gpsimd.collective_compute(
                "ReduceScatter", OP.add, replica_groups=rg,
                ins=[p2b[:].opt()], outs=[rs2[:].opt()],
            )

            # ---------------- layer-2 post + output -------------------------
            for mt in range(NMT):
                r2t = io.tile([128, 512], f16, tag="r2t", bufs=2)
                nc.sync.dma_start(r2t[:], rs2[mt * 128:(mt + 1) * 128, :])
                z1 = io.tile([128, 512], f32, tag="z1", bufs=2)
                nc.vector.scalar_tensor_tensor(
                    out=z1[:], in0=y2s[:, mt, :], scalar=dl4[:, mt:mt + 1],
                    in1=r2t[:], op0=OP.mult, op1=OP.add)
                nc.vector.scalar_tensor_tensor(
                    out=z1[:], in0=z1[:], scalar=dis4[:, mt:mt + 1],
                    in1=b1bc[:], op0=OP.mult, op1=OP.add)
                zo = io.tile([128, 512], f16, tag="zo", bufs=2)
                nc.scalar.activation(zo[:], z1[:], AF.Relu)
                nc.sync.dma_start(ZOUT[mt * 128:(mt + 1) * 128, :], zo[:])
    return nc


def _run(name, builder, in_maps):
    global _LAST_DEVICE_WALL
    if name not in _CACHE:
        _CACHE[name] = builder()
    t0 = time.time()
    res = run_bass_kernel_spmd(_CACHE[name], in_maps, core_ids=list(range(NC)))
    dt = time.time() - t0
    _LAST_DEVICE_WALL += dt
    import os
    if os.environ.get("KERNEL_DEBUG_TIMES"):
        print(f"  [dev call {name}: {dt * 1e3:.1f} ms]")
    return res.results


# ---------------------------------------------------------------------------
def _l2n(x):
    n = np.sqrt(np.sum(x * x, axis=1, keepdims=True))
    return x / np.maximum(n, 1e-12)


def kernel(x, new_edge, beta, delta, eps, Wm, bm, Ws, bs,
           mW0, mb0, mW1, mb1, sW0, sb0, sW1, sb1):
    global _LAST_DEVICE_WALL
    _LAST_DEVICE_WALL = 0.0
    x = np.asarray(x, np.float32)
    b = float(np.asarray(beta).reshape(-1)[0])
    d = float(np.asarray(delta).reshape(-1)[0])

    x_mean = x @ Wm + bm
    x_std = x @ Ws + bs

    m = _l2n(x_mean)
    c = _l2n(np.exp(x_std))
    cs = np.sqrt(c)
    sq = np.sum(m * m, axis=1)
    csum = np.sum(c, axis=1)
    u = (sq + csum).astype(np.float32)

    g = np.concatenate([m, cs], axis=1).astype(np.float32)  # [N, 1024]

    ne = np.asarray(new_edge, np.float32)
    NEBq = np.round(ne * 255.0).astype(np.uint8)       # dequant scale b/255
    Eq = np.round(np.asarray(eps, np.float32) * 65535.0).astype(np.uint16)

    WBD = np.zeros((1024, 512), np.float16)
    WBD[:512, :256] = mW1
    WBD[512:, 256:] = sW1

    BRv = np.concatenate([mb0, sb0, mb1, sb1]).astype(np.float16).reshape(1, 1536)
    # packed projection weights + folded layer-0 weights / bias rows
    WCAT = np.concatenate(
        [Wm, Ws, Wm @ mW0, Ws @ sW0], axis=1).astype(np.float16)  # [512, 2048]
    BR2v = np.concatenate(
        [bm, bs, bm @ mW0, bs @ sW0]).astype(np.float32).reshape(1, 2048)
    x16 = x.astype(np.float16)
    XI_cat = np.ascontiguousarray(
        x16.reshape(NC, BLK, F).transpose(0, 2, 1).reshape(NC * F, BLK))

    # host diagonal scalars (match the on-device dataflow, f32 approx)
    g2 = np.sum(g * g, axis=1)
    wsdiag = np.exp(-(2.0 * u - 2.0 * g2)).astype(np.float32)
    nebdiag = np.diagonal(NEBq).astype(np.float32) * (b / 255.0)
    ed = np.clip(np.diagonal(Eq).astype(np.float32) / 65535.0,
                 EPS_CLAMP, 1.0 - EPS_CLAMP)
    Ediag = ed / (1.0 - ed)

    # pre-concatenated (all-cores) inputs: axis0 = core-major, zero extra
    # copies for the row-sharded tensors
    SM_cat = np.zeros((NC, 128, 16), np.float32)
    SM_cat[:, :, 0:4] = wsdiag.reshape(NC, 4, 128).transpose(0, 2, 1)
    SM_cat[:, :, 4:8] = nebdiag.reshape(NC, 4, 128).transpose(0, 2, 1)
    SM_cat[:, :, 8:12] = Ediag.reshape(NC, 4, 128).transpose(0, 2, 1)
    SM_cat[:, :, 12] = -d
    SM_cat[:, :, 13] = 1.0 - b
    SM_cat[:, :, 14] = b / 255.0
    SM_cat = SM_cat.reshape(NC * 128, 16)
    concat_map = {
        "XI": XI_cat,
        "WCK": WCAT,
        "BR2": np.repeat(BR2v, NC, axis=0),
        "NEBQ": NEBq,
        "EQ": Eq,
        "WBK": WBD,
        "BR": np.repeat(BRv, NC, axis=0),
        "SM": SM_cat,
    }

    res = _run("fused", _build_fused, [{"__concat__": concat_map}])
    z = np.concatenate([res[k]["ZOUT"] for k in range(NC)], axis=0)
    z_mean = np.ascontiguousarray(z[:, :H]).astype(np.float32)
    z_std = np.ascontiguousarray(z[:, H:2 * H]).astype(np.float32)
    return z_mean, z_std
